# revision 1
# baseline (speedup 1.0000x reference)
"""Trainium2 Bass kernel for nn_BaseAttention (B=2, N=2048, E=2048, H=16, D=128).

Sharding: 8 cores; core c handles batch b=c//4, head-group hg=c%4 (4 heads).
Each core computes q/k/v projections for its heads, causal flash-style
attention, and a partial out-projection (contraction over its 512 head dims).
Host sums the 4 partial outputs per batch (tensor-parallel unshard).

Projections and out-projection run as float32r (full PE rate at free dim >=
256). q/k spill to DRAM as bf16, so QK^T and A@V are bf16 matmuls. exp runs
on ScalarE straight out of PSUM (only the causally-valid region); softmax
denominators use a DVE pairwise tree + ones-matmul partition reduction, a
K=1 broadcast matmul, and the fast approximate reciprocal.
"""

import os
import sys
import time

sys.path.insert(0, "/opt/trn_rl_repo")

PHASES = {"proj", "attn", "norm", "oproj"}

import numpy as np
import ml_dtypes

import concourse.bass as bass
import concourse.mybir as mybir
import concourse.tile as tile
from concourse import bacc
from concourse.bass_utils import run_bass_kernel_spmd

B, N, E, H = 2, 2048, 2048, 16
D = E // H            # 128
HPC = 4               # heads per core
DC = HPC * D          # 512 head dims per core
NCORES = 8
P = 128
NCH = N // 512        # 4 n-chunks of 512
ET = E // P           # 16 e-tiles of 128

F32 = mybir.dt.float32
F32R = mybir.dt.float32r
BF16 = mybir.dt.bfloat16
FP16 = mybir.dt.float16


def build_nc():
    nc = bacc.Bacc("TRN2", target_bir_lowering=False, debug=False,
                   num_devices=NCORES)

    xT = nc.dram_tensor("xT", [E, N], FP16, kind="ExternalInput")
    wqT = nc.dram_tensor("wqT", [E, DC], FP16, kind="ExternalInput")
    wkT = nc.dram_tensor("wkT", [E, DC], FP16, kind="ExternalInput")
    wvT = nc.dram_tensor("wvT", [E, DC], FP16, kind="ExternalInput")
    woT = nc.dram_tensor("woT", [DC, E], FP16, kind="ExternalInput")
    maskin = nc.dram_tensor("maskin", [P, 4, 512], FP16, kind="ExternalInput")
    out = nc.dram_tensor("out", [N, E], F32, kind="ExternalOutput")

    xT_r = xT.ap().rearrange("(eo p) n -> p eo n", p=P)      # [128,16,2048]
    wqT_r = wqT.ap().rearrange("(eo p) d -> p eo d", p=P)    # [128,16,512]
    wkT_r = wkT.ap().rearrange("(eo p) d -> p eo d", p=P)
    wvT_r = wvT.ap().rearrange("(eo p) d -> p eo d", p=P)
    woT_r = woT.ap().rearrange("(t p) e -> p t e", p=P)      # [128,4,2048]

    with tile.TileContext(nc) as tc:
        # ---------------- constants + spill tensors ----------------
        consts = tc.alloc_tile_pool(name="consts", bufs=1)
        _longlived = [consts]
        mask_sb = consts.tile([P, 4, 512], FP16)
        # prefire the Exp table load so it overlaps the input DMA head
        dummy = consts.tile([1, 8], F32)
        nc.vector.memset(dummy, 0.0)
        nc.scalar.activation(out=dummy, in_=dummy,
                             func=mybir.ActivationFunctionType.Exp)

        dram = tc.alloc_tile_pool(name="dram", bufs=1, space="DRAM")
        _longlived.append(dram)
        attd = dram.tile([HPC, N, D], FP16)          # normalized attn out

        # per-core activations, SBUF-resident across the whole kernel
        big = tc.alloc_tile_pool(name="big", bufs=1)
        _longlived.append(big)
        qs = big.tile([P, HPC, N], FP16)                  # q^T, heads stacked
        ks = big.tile([P, HPC, N], FP16)                  # k^T
        v_all = big.tile([P, N // P, HPC, D + 4], FP16)   # [V | 1] per block
        nc.vector.memset(v_all[:, :, :, D:D + 1], 1.0)

        # ---------------- phase 1: q/k/v projections ----------------
        if "proj" in PHASES:
         with (
            tc.tile_pool(name="wpool", bufs=1) as wpool,
            tc.tile_pool(name="xpool", bufs=2) as xpool,
            tc.tile_pool(name="pj_ps", bufs=4, space="PSUM") as pj_ps,
         ):
            wq_sb = wpool.tile([P, ET, DC], FP16)
            wk_sb = wpool.tile([P, ET, DC], FP16)
            wv_sb = wpool.tile([P, ET, DC], FP16)
            x_tiles = [None] * NCH

            def load_x(nch):
                t = xpool.tile([P, ET, 512], FP16, tag="xchunk",
                               name=f"x_sb{nch}")
                nc.sync.dma_start(
                    out=t, in_=xT_r[:, :, nch * 512:(nch + 1) * 512])
                x_tiles[nch] = t

            # interleaved preload in 1MB pieces: PE starts after the first
            # wq piece + x0 piece; DMA then feeds just-in-time
            QT = ET // 4
            x0 = xpool.tile([P, ET, 512], FP16, tag="xchunk", name="x_sb0")
            x_tiles[0] = x0
            # extra-fine first pieces so the first matmul starts earlier
            for g2, (a, b) in enumerate(((0, 1), (1, 2), (2, 4))):
                gs = slice(a, b)
                nc.sync.dma_start(out=wq_sb[:, gs, :], in_=wqT_r[:, gs, :])
                nc.sync.dma_start(out=x0[:, gs, :],
                                  in_=xT_r[:, gs, 0:512])
            for g in range(1, 4):
                gs = slice(g * QT, (g + 1) * QT)
                nc.sync.dma_start(out=wq_sb[:, gs, :], in_=wqT_r[:, gs, :])
                nc.sync.dma_start(out=x0[:, gs, :],
                                  in_=xT_r[:, gs, 0:512])
            for g in range(4):
                gs = slice(g * QT, (g + 1) * QT)
                nc.sync.dma_start(out=wk_sb[:, gs, :], in_=wkT_r[:, gs, :])
            load_x(1)
            nc.sync.dma_start(out=mask_sb, in_=maskin.ap())
            HF = ET // 2
            nc.sync.dma_start(out=wv_sb[:, :HF, :], in_=wvT_r[:, :HF, :])
            nc.sync.dma_start(out=wv_sb[:, HF:, :], in_=wvT_r[:, HF:, :])

            for nch in range(NCH):
                x_sb = x_tiles[nch]
                nsl = slice(nch * 512, (nch + 1) * 512)

                # qT / kT: psum[dq_tile 128, 512 n] = sum_e W[e, dq] x[e, n]
                for w_sb, dst in ((wq_sb, qs), (wk_sb, ks)):
                    for t in range(HPC):
                        ps = pj_ps.tile([P, 512], F32, tag="pjps")
                        for et in range(ET):
                            nc.tensor.matmul(
                                ps,
                                lhsT=w_sb[:, et, t * P:(t + 1) * P],
                                rhs=x_sb[:, et, :],
                                start=(et == 0), stop=(et == ET - 1),
                            )
                        if nch == NCH - 1:
                            nc.scalar.copy(out=dst[:, t, nsl], in_=ps)
                        else:
                            nc.vector.tensor_copy(out=dst[:, t, nsl], in_=ps)

                # v: psum[n_block 128, 512 dv] = sum_e x[e, n] Wv[e, dv]
                for nb in range(4):
                    ps = pj_ps.tile([P, 512], F32, tag="pjps")
                    for et in range(ET):
                        nc.tensor.matmul(
                            ps,
                            lhsT=x_sb[:, et, nb * P:(nb + 1) * P],
                            rhs=wv_sb[:, et, :],
                            start=(et == 0), stop=(et == ET - 1),
                        )
                    if nch == NCH - 1:
                        nc.scalar.copy(
                            out=v_all[:, nch * 4 + nb, :, :D],
                            in_=ps.rearrange("p (h d) -> p h d", h=HPC))
                    else:
                        nc.vector.tensor_copy(
                            out=v_all[:, nch * 4 + nb, :, :D],
                            in_=ps.rearrange("p (h d) -> p h d", h=HPC))

                if nch + 2 < NCH:
                    load_x(nch + 2)

        # ---------------- phase 2: attention ----------------
        outT_pool = tc.alloc_tile_pool(name="outT", bufs=1)
        _longlived.append(outT_pool)
        outTs = [outT_pool.tile([P, N], FP16, name=f"outT{t}")
                 for t in range(HPC)]
        wo_pool = tc.alloc_tile_pool(name="wo_pool", bufs=1)
        _longlived.append(wo_pool)
        wo_sb = wo_pool.tile([P, HPC, E], FP16)
        if "oproj" in PHASES and "attn" not in PHASES:
            for t in range(HPC):
                nc.sync.dma_start(out=wo_sb[:, t, :], in_=woT_r[:, t, :])

        if "attn" in PHASES:
         with (
            tc.tile_pool(name="pt_pool", bufs=3) as pt_pool,
            tc.tile_pool(name="att_pool", bufs=3) as att_pool,
            tc.tile_pool(name="rs_pool", bufs=8) as rs_pool,
            tc.tile_pool(name="qk_ps", bufs=2, space="PSUM") as qk_ps,
            tc.tile_pool(name="av_ps", bufs=4, space="PSUM") as av_ps,
         ):
            for h in range(HPC):
                if "oproj" in PHASES:
                    nc.sync.dma_start(out=wo_sb[:, h, :], in_=woT_r[:, h, :])
                att_h = att_pool.tile([P, N // P, D], FP16, tag="atth")

                for ci in range(NCH):
                    BJ = 4 * (ci + 1)
                    pt = pt_pool.tile([P, ET, 512], FP16, tag="pt")
                    if h == 0:
                        # first use of each slice range of the rotating pool:
                        # clear the regions partial-exp never writes so the
                        # mask multiply sees finite values, not NaN garbage
                        nc.vector.memset(pt[:, BJ - 2, :256], 0.0)
                        nc.vector.memset(pt[:, BJ - 1, :384], 0.0)
                    # scores^T tiles [j_block, i_chunk] + exp (2 tiles/ACT op)
                    for bjp in range(BJ // 2):
                        ps = qk_ps.tile([P, 2, 512], F32, tag="qkps")
                        last_pair = (bjp == BJ // 2 - 1)
                        for u in range(2):
                            bj = 2 * bjp + u
                            nc.tensor.matmul(
                                ps[:, u, :],
                                lhsT=ks[:, h, bj * P:(bj + 1) * P],
                                rhs=qs[:, h, ci * 512:(ci + 1) * 512],
                                start=True, stop=True,
                            )
                        if last_pair:
                            # diagonal blocks r=256,384: only cols >= r valid
                            nc.scalar.activation(
                                out=pt[:, 2 * bjp, 256:], in_=ps[:, 0, 256:],
                                func=mybir.ActivationFunctionType.Exp,
                            )
                            nc.scalar.activation(
                                out=pt[:, 2 * bjp + 1, 384:], in_=ps[:, 1, 384:],
                                func=mybir.ActivationFunctionType.Exp,
                            )
                        else:
                            nc.scalar.activation(
                                out=pt[:, 2 * bjp:2 * bjp + 2, :], in_=ps,
                                func=mybir.ActivationFunctionType.Exp,
                            )
                    # causal masks on the diagonal blocks (bj = BJ-4 .. BJ-1)
                    # full-tile: the mask's zero prefix also clears regions
                    # exp never wrote (stale finite values from pool reuse)
                    for rr in range(4):
                        bj = BJ - 4 + rr
                        nc.vector.tensor_mul(
                            out=pt[:, bj, :], in0=pt[:, bj, :],
                            in1=mask_sb[:, rr, :])

                    # A @ [V | 1]: out rows are queries, col 128 is the
                    # softmax denominator; normalize on eviction
                    for ib in range(4):
                        avp = av_ps.tile([P, D + 4], F32, tag="avps")
                        isl = slice(ib * P, (ib + 1) * P)
                        for bj in range(BJ):
                            nc.tensor.matmul(
                                avp[:, :D + 1],
                                lhsT=pt[:, bj, isl],
                                rhs=v_all[:, bj, h, :D + 1],
                                start=(bj == 0), stop=(bj == BJ - 1),
                            )
                        rs = rs_pool.tile([P, 1], F32, tag="rs")
                        nc.vector.reciprocal_approx_fast(
                            out=rs, in_=avp[:, D:D + 1])
                        nc.vector.tensor_scalar_mul(
                            out=att_h[:, ci * 4 + ib, :], in0=avp[:, :D],
                            scalar1=rs)

                    # spill + transpose this ci's slice right away so the
                    # out-projection isn't gated on the whole head
                    csl = slice(ci * 4, (ci + 1) * 4)
                    nc.sync.dma_start(
                        out=attd[h, ci * 512:(ci + 1) * 512, :].rearrange(
                            "(io p) d -> p io d", p=P),
                        in_=att_h[:, csl, :])
                    nc.sync.dma_start_transpose(
                        out=outTs[h][:, ci * 512:(ci + 1) * 512],
                        in_=attd[h, ci * 512:(ci + 1) * 512, :])

        # ---------------- phase 4: out projection (partial) ----------------
        if "oproj" in PHASES:
         with (
            tc.tile_pool(name="op_ps", bufs=4, space="PSUM") as op_ps,
            tc.tile_pool(name="op_ev", bufs=3) as op_ev,
         ):
            for nb in range(N // P):
                ostage = op_ev.tile([P, NCH, 512], F32, tag="opev")
                for ec in range(NCH):
                    ps = op_ps.tile([P, 512], F32, tag="opps")
                    for t in range(HPC):
                        nc.tensor.matmul(
                            ps,
                            lhsT=outTs[t][:, nb * P:(nb + 1) * P],
                            rhs=wo_sb[:, t, ec * 512:(ec + 1) * 512],
                            start=(t == 0), stop=(t == HPC - 1),
                        )
                    nc.any.tensor_copy(out=ostage[:, ec, :], in_=ps)
                nc.sync.dma_start(
                    out=out.ap()[nb * P:(nb + 1) * P, :], in_=ostage)

        for _pl in reversed(_longlived):
            _pl.release()

    nc.compile()
    return nc


def make_in_maps(x, Wq, Wkv, Wout):
    x = np.asarray(x, dtype=np.float32)
    Wq = np.asarray(Wq, dtype=np.float32)
    Wkv = np.asarray(Wkv, dtype=np.float32)
    Wout = np.asarray(Wout, dtype=np.float32)
    scale = np.float32(D ** -0.5)

    # causal masks for the 4 diagonal offsets
    jj = np.arange(P)[:, None]
    ii = np.arange(512)[None, :]
    mask = np.zeros((P, 4, 512), dtype=np.float16)
    for rr in range(4):
        mask[:, rr, :] = (ii >= jj + rr * P).astype(np.float16)

    xT = [np.ascontiguousarray(x[b].T).astype(np.float16) for b in range(B)]
    in_maps = []
    for c in range(NCORES):
        b, hg = divmod(c, 4)
        sl = slice(hg * DC, (hg + 1) * DC)
        in_maps.append({
            "xT": xT[b],
            "wqT": (np.ascontiguousarray(Wq[sl, :].T) * scale).astype(np.float16),
            "wkT": np.ascontiguousarray(Wkv[sl, :].T).astype(np.float16),
            "wvT": np.ascontiguousarray(Wkv[E + sl.start:E + sl.stop, :].T).astype(np.float16),
            "woT": np.ascontiguousarray(Wout[:, sl].T).astype(np.float16),
            "maskin": mask,
        })
    return in_maps


_NC_CACHE = []


def _get_nc():
    if not _NC_CACHE:
        _NC_CACHE.append(build_nc())
    return _NC_CACHE[0]


def _run(in_maps):
    nc = _get_nc()
    return run_bass_kernel_spmd(nc, in_maps, core_ids=list(range(NCORES)))


def kernel(x, Wq, Wkv, Wout):
    in_maps = make_in_maps(x, Wq, Wkv, Wout)
    res = _run(in_maps)
    out = np.zeros((B, N, E), dtype=np.float32)
    for c in range(NCORES):
        out[c // 4] += res.results[c]["out"]
    return out


if __name__ == "__main__":
    t0 = time.time()
    _get_nc()
    print(f"build+compile: {time.time() - t0:.1f}s")



# revision 26
# speedup vs baseline: 1.0789x; 1.0789x over previous
"""Trainium2 Bass kernel for nn_BaseAttention (B=2, N=2048, E=2048, H=16, D=128).

Sharding: 8 cores; core c handles batch b=c//4, head-group hg=c%4 (4 heads).
Each core computes q/k/v projections for its heads, causal flash-style
attention, and a partial out-projection (contraction over its 512 head dims).
Host sums the 4 partial outputs per batch (tensor-parallel unshard).

Schedule (v3):
- QK^T and A@V are causally exact at 128-block granularity; only the
  strictly-diagonal 128x128 squares get a triangular mask multiply.
- The four ci=0 attention tiles (which need only chunk 0 of q/k/v) are woven
  into the projection phase, so their exp/spill/transpose chain finishes long
  before the attention phase starts.
- In the attention phase, out-projection matmuls and the previous tile's A@V
  are emitted as PE fill work BEFORE each QK PSUM pair, so the in-order PE
  never parks on the ScalarE exp cadence.
- The last attention row's outputs are transposed on the PE (via identity
  matmul) instead of the DRAM round-trip, removing the final transpose DMA
  latency from the critical path.
- A dummy warmup matmul chain absorbs the PE p-state ramp while the first
  input DMA pieces land; the first projection chunk runs et-outer so the PE
  starts consuming pieces as they arrive.
- Output is written fp16 and summed on host in fp32.
"""

import sys
import time

sys.path.insert(0, "/opt/trn_rl_repo")

from collections import deque

import numpy as np

import concourse.bass as bass
import concourse.mybir as mybir
import concourse.tile as tile
from concourse import bacc
from concourse.bass_utils import run_bass_kernel_spmd

B, N, E, H = 2, 2048, 2048, 16
D = E // H            # 128
HPC = 4               # heads per core
DC = HPC * D          # 512 head dims per core
NCORES = 8
P = 128
NCH = N // 512        # 4 n-chunks of 512
ET = E // P           # 16 e-tiles of 128

F32 = mybir.dt.float32
FP16 = mybir.dt.float16


def build_nc():
    nc = bacc.Bacc("TRN2", target_bir_lowering=False, debug=False,
                   num_devices=NCORES)

    xT = nc.dram_tensor("xT", [E, N], FP16, kind="ExternalInput")
    wqT = nc.dram_tensor("wqT", [E, DC], FP16, kind="ExternalInput")
    wkT = nc.dram_tensor("wkT", [E, DC], FP16, kind="ExternalInput")
    wvT = nc.dram_tensor("wvT", [E, DC], FP16, kind="ExternalInput")
    woT = nc.dram_tensor("woT", [DC, E], FP16, kind="ExternalInput")
    maskin = nc.dram_tensor("maskin", [P, 2, P], FP16, kind="ExternalInput")
    out = nc.dram_tensor("out", [N, E], FP16, kind="ExternalOutput")

    xT_r = xT.ap().rearrange("(eo p) n -> p eo n", p=P)      # [128,16,2048]
    wqT_r = wqT.ap().rearrange("(eo p) d -> p eo d", p=P)    # [128,16,512]
    wkT_r = wkT.ap().rearrange("(eo p) d -> p eo d", p=P)
    wvT_r = wvT.ap().rearrange("(eo p) d -> p eo d", p=P)
    woT_r = woT.ap().rearrange("(t p) e -> p t e", p=P)      # [128,4,2048]

    EXPF = mybir.ActivationFunctionType.Exp

    with tile.TileContext(nc) as tc:
        # ---------------- constants + spill tensors ----------------
        consts = tc.alloc_tile_pool(name="consts", bufs=1)
        _longlived = [consts]
        mask_sb = consts.tile([P, 2, P], FP16)   # [tri(c>=p) | identity]
        warm_sb = consts.tile([P, 512], FP16)
        # prefire the Exp table load so it overlaps the input DMA head
        dummy = consts.tile([1, 8], F32)
        nc.vector.memset(dummy, 0.0)
        nc.scalar.activation(out=dummy, in_=dummy, func=EXPF)
        nc.vector.memset(warm_sb, 0.0)

        # per-core activations, SBUF-resident across the whole kernel
        big = tc.alloc_tile_pool(name="big", bufs=1)
        _longlived.append(big)
        qs = big.tile([P, HPC, N], FP16)                  # q^T, heads stacked
        ks = big.tile([P, HPC, N], FP16)                  # k^T
        v_all = big.tile([P, N // P, HPC, D + 4], FP16)   # [V | 1] per block
        nc.vector.memset(v_all[:, :, :, D:D + 1], 1.0)

        outT_pool = tc.alloc_tile_pool(name="outT", bufs=1)
        _longlived.append(outT_pool)
        outTs = [outT_pool.tile([P, N], FP16, name=f"outT{t}")
                 for t in range(HPC)]
        wo_pool = tc.alloc_tile_pool(name="wo_pool", bufs=1)
        _longlived.append(wo_pool)
        wo_sb = wo_pool.tile([P, HPC, E], FP16)

        pt_pool = tc.alloc_tile_pool(name="pt_pool", bufs=2)
        att_pool = tc.alloc_tile_pool(name="att_pool", bufs=3)
        rs_pool = tc.alloc_tile_pool(name="rs_pool", bufs=8)

        # ---------------- phase 1a: nch-0 projections (et-outer) --------
        wpool = tc.alloc_tile_pool(name="wpool", bufs=1)
        xpool = tc.alloc_tile_pool(name="xpool", bufs=2)
        wq_sb = wpool.tile([P, ET, DC], FP16)
        wk_sb = wpool.tile([P, ET, DC], FP16)
        wv_sb = wpool.tile([P, ET, DC], FP16)
        x_tiles = [None] * NCH

        def load_x(nch):
            t = xpool.tile([P, ET, 512], FP16, tag="xchunk",
                           name=f"x_sb{nch}")
            nc.sync.dma_start(
                out=t, in_=xT_r[:, :, nch * 512:(nch + 1) * 512])
            x_tiles[nch] = t

        with tc.tile_pool(name="pj8", bufs=8, space="PSUM") as pj8:
            # PE warmup: absorb the p-state ramp on dummy matmuls while the
            # first input pieces stream in
            warm_ps = pj8.tile([P, 512], F32, tag="pjps")
            for w in range(8):
                nc.tensor.matmul(warm_ps, lhsT=warm_sb[:, 0:P],
                                 rhs=warm_sb, start=(w == 0), stop=(w == 7))

            # small pieces throughout: PE consumption (~0.85us/et) only just
            # trails DMA supply (~0.72us/et), so a late big piece stalls PE
            x0 = xpool.tile([P, ET, 512], FP16, tag="xchunk", name="x_sb0")
            x_tiles[0] = x0
            for a, b in ((0, 1), (1, 2), (2, 4), (4, 6), (6, 8), (8, 10),
                         (10, 12), (12, 14), (14, 16)):
                gs = slice(a, b)
                nc.sync.dma_start(out=wq_sb[:, gs, :], in_=wqT_r[:, gs, :])
                nc.sync.dma_start(out=x0[:, gs, :], in_=xT_r[:, gs, 0:512])
            for g in range(4):
                gs = slice(g * 4, (g + 1) * 4)
                nc.sync.dma_start(out=wk_sb[:, gs, :], in_=wkT_r[:, gs, :])
            load_x(1)
            nc.sync.dma_start(out=mask_sb, in_=maskin.ap())
            HF = ET // 2
            nc.sync.dma_start(out=wv_sb[:, :HF, :], in_=wvT_r[:, :HF, :])
            nc.sync.dma_start(out=wv_sb[:, HF:, :], in_=wvT_r[:, HF:, :])

            for w_sb, dst in ((wq_sb, qs), (wk_sb, ks)):
                pss = [pj8.tile([P, 512], F32, tag="pjps", name=f"pjt{t}")
                       for t in range(HPC)]
                for et in range(ET):
                    for t in range(HPC):
                        nc.tensor.matmul(
                            pss[t],
                            lhsT=w_sb[:, et, t * P:(t + 1) * P],
                            rhs=x0[:, et, :],
                            start=(et == 0), stop=(et == ET - 1),
                        )
                for t in range(HPC):
                    nc.vector.tensor_copy(out=dst[:, t, 0:512], in_=pss[t])
            for nb in range(4):
                ps = pj8.tile([P, 512], F32, tag="pjps")
                for et in range(ET):
                    nc.tensor.matmul(
                        ps,
                        lhsT=x0[:, et, nb * P:(nb + 1) * P],
                        rhs=wv_sb[:, et, :],
                        start=(et == 0), stop=(et == ET - 1),
                    )
                nc.vector.tensor_copy(
                    out=v_all[:, nb, :, :D],
                    in_=ps.rearrange("p (h d) -> p h d", h=HPC))
            load_x(2)
            # wo loads here (DMA slack mid-proj) so the attention-phase DMA
            # queue is free for the attention spills/transposes
            for t in range(HPC):
                nc.sync.dma_start(out=wo_sb[:, t, :], in_=woT_r[:, t, :])

        # ------- phase 1b + 2: proj nch 1-3 (with ci=0 attn tiles woven
        # in), then attention rows 1-3 merged with the out-projection -----
        if True:
            psum_pools = {}
            tri = mask_sb[:, 0, :]
            ident = mask_sb[:, 1, :]

            # ---- attention tile helpers (used for ci=0 during proj and
            # for rows 1..3 in the attention phase) ----
            def emit_qk_pair(ci, h, pt, pi):
                npairs = 2 * ci + 2
                bj0, bj1 = 2 * pi, 2 * pi + 1
                ps = psum_pools["qk"].tile([P, 2, 512], F32, tag="qkps")
                for u, bj in ((0, bj0), (1, bj1)):
                    rr = bj - 4 * ci
                    if rr <= 0:
                        nc.tensor.matmul(
                            ps[:, u, :],
                            lhsT=ks[:, h, bj * P:(bj + 1) * P],
                            rhs=qs[:, h, ci * 512:(ci + 1) * 512],
                            start=True, stop=True,
                        )
                    else:
                        nc.tensor.matmul(
                            ps[:, u, rr * P:],
                            lhsT=ks[:, h, bj * P:(bj + 1) * P],
                            rhs=qs[:, h, ci * 512 + rr * P:(ci + 1) * 512],
                            start=True, stop=True,
                        )
                if pi == npairs - 1:
                    # diagonal pair: only causally-valid columns
                    nc.scalar.activation(
                        out=pt[:, bj0, 256:], in_=ps[:, 0, 256:], func=EXPF)
                    nc.scalar.activation(
                        out=pt[:, bj1, 384:], in_=ps[:, 1, 384:], func=EXPF)
                else:
                    nc.scalar.activation(
                        out=pt[:, bj0:bj0 + 2, :], in_=ps, func=EXPF)
                # triangular mask on strictly-diagonal 128x128 squares
                for u, bj in ((0, bj0), (1, bj1)):
                    rr = bj - 4 * ci
                    if rr >= 0:
                        sq = slice(rr * P, (rr + 1) * P)
                        nc.vector.tensor_mul(
                            out=pt[:, bj, sq], in0=pt[:, bj, sq], in1=tri)

            def av_item(ci, h, ib, pt, att_h):
                gi = 4 * ci + ib
                avp = psum_pools["av"].tile([P, D + 4], F32, tag="avps")
                isl = slice(ib * P, (ib + 1) * P)
                for bj in range(gi + 1):
                    nc.tensor.matmul(
                        avp[:, :D + 1],
                        lhsT=pt[:, bj, isl],
                        rhs=v_all[:, bj, h, :D + 1],
                        start=(bj == 0), stop=(bj == gi),
                    )
                rs = rs_pool.tile([P, 1], F32, tag="rs")
                nc.vector.reciprocal_approx_fast(out=rs, in_=avp[:, D:D + 1])
                nc.vector.tensor_scalar_mul(
                    out=att_h[:, ib, :], in0=avp[:, :D], scalar1=rs)
                # transpose on the PE (identity matmul): no DRAM round
                # trip, so out-proj groups unlock right after the eviction
                tp = psum_pools["av"].tile([P, D], FP16, tag="avps", name="tps")
                nc.tensor.transpose(tp, att_h[:, ib, :], ident)
                nc.vector.tensor_copy(
                    out=outTs[h][:, ci * 512 + ib * P:
                                 ci * 512 + (ib + 1) * P],
                    in_=tp)

            # ---- proj nch 1..3 with ci=0 tiles woven between chains ----
            with (
                tc.tile_pool(name="pj2", bufs=3, space="PSUM") as pj2,
                tc.tile_pool(name="qkw", bufs=1, space="PSUM") as qkw,
                tc.tile_pool(name="avw", bufs=1, space="PSUM") as avw,
            ):
                psum_pools["qk"] = qkw
                psum_pools["av"] = avw
                pts0 = [pt_pool.tile([P, ET, 512], FP16, tag="pt",
                                     name=f"pt0{h}") for h in range(HPC)]
                atts0 = [att_pool.tile([P, NCH, D], FP16, tag="atth",
                                       name=f"att0{h}") for h in range(HPC)]
                # (kind, args): proj chain steps interleaved with ci=0 work
                weave = deque()
                for h in range(HPC):
                    weave.append(("qk", h, 0))
                    weave.append(("qk", h, 1))
                    for ib in range(4):
                        weave.append(("av", h, ib))

                def weave_step(budget):
                    # pop ci=0 attn pieces; each is tiny vs a proj chain
                    n = 0
                    while weave and n < budget:
                        kind, h, idx = weave[0]
                        if kind == "qk":
                            emit_qk_pair(0, h, pts0[h], idx)
                        else:
                            av_item(0, h, idx, pts0[h], atts0[h])
                        weave.popleft()
                        n += 1

                for nch in range(1, NCH):
                    x_sb = x_tiles[nch]
                    nsl = slice(nch * 512, (nch + 1) * 512)

                    for w_sb, dst in ((wq_sb, qs), (wk_sb, ks)):
                        for t in range(HPC):
                            ps = pj2.tile([P, 512], F32, tag="pjps")
                            for et in range(ET):
                                nc.tensor.matmul(
                                    ps,
                                    lhsT=w_sb[:, et, t * P:(t + 1) * P],
                                    rhs=x_sb[:, et, :],
                                    start=(et == 0), stop=(et == ET - 1),
                                )
                            if nch == NCH - 1:
                                nc.scalar.copy(out=dst[:, t, nsl], in_=ps)
                            else:
                                nc.vector.tensor_copy(
                                    out=dst[:, t, nsl], in_=ps)
                            weave_step(1)

                    for nb in range(4):
                        ps = pj2.tile([P, 512], F32, tag="pjps")
                        for et in range(ET):
                            nc.tensor.matmul(
                                ps,
                                lhsT=x_sb[:, et, nb * P:(nb + 1) * P],
                                rhs=wv_sb[:, et, :],
                                start=(et == 0), stop=(et == ET - 1),
                            )
                        if nch == NCH - 1:
                            nc.scalar.copy(
                                out=v_all[:, nch * 4 + nb, :, :D],
                                in_=ps.rearrange("p (h d) -> p h d", h=HPC))
                        else:
                            nc.vector.tensor_copy(
                                out=v_all[:, nch * 4 + nb, :, :D],
                                in_=ps.rearrange("p (h d) -> p h d", h=HPC))
                        weave_step(1)

                    if nch + 2 < NCH:
                        load_x(nch + 2)
                weave_step(99)
            xpool.release()
            wpool.release()

            # ---- attention rows 1..3 + out-projection fill ----
            with (
                tc.tile_pool(name="qk_ps", bufs=2, space="PSUM") as qk_ps,
                tc.tile_pool(name="av_ps", bufs=2, space="PSUM") as av_ps,
                tc.tile_pool(name="op_ps", bufs=2, space="PSUM") as op_ps,
                tc.tile_pool(name="op_ev", bufs=3) as op_ev,
            ):
                psum_pools["qk"] = qk_ps
                psum_pools["av"] = av_ps
                favq = deque()     # (cost_ns, emit_fn) A@V of the prev tile
                fopq = deque()     # (cost_ns, emit_fn) out-proj items
                transposed = [HPC, 0, 0, 0]
                tdone_step = [-99, None, None, None]
                op_queued = [False] * NCH
                op_state = {}      # nb -> ostage tile
                evict_flip = [0]
                step = [0]

                def make_op_item(nb, ec):
                    def emit():
                        if ec == 0:
                            op_state[nb] = op_ev.tile(
                                [P, NCH, 512], FP16, tag="opev",
                                name=f"ost{nb}")
                        ostage = op_state[nb]
                        ps = op_ps.tile([P, 512], F32, tag="opps")
                        for t in range(HPC):
                            nc.tensor.matmul(
                                ps,
                                lhsT=outTs[t][:, nb * P:(nb + 1) * P],
                                rhs=wo_sb[:, t, ec * 512:(ec + 1) * 512],
                                start=(t == 0), stop=(t == HPC - 1),
                            )
                        if evict_flip[0] == 0:
                            nc.vector.tensor_copy(out=ostage[:, ec, :],
                                                  in_=ps)
                        else:
                            nc.scalar.copy(out=ostage[:, ec, :], in_=ps)
                        evict_flip[0] ^= 1
                        if nb == 4 * NCH - 1:
                            # very last row-block: per-ec DMAs so the final
                            # transfer trailing the last matmul is small
                            nc.sync.dma_start(
                                out=out.ap()[nb * P:(nb + 1) * P,
                                             ec * 512:(ec + 1) * 512],
                                in_=ostage[:, ec, :])
                        elif nb >= 4 * (NCH - 1):
                            if ec == 1:
                                nc.sync.dma_start(
                                    out=out.ap()[nb * P:(nb + 1) * P,
                                                 0:1024],
                                    in_=ostage[:, 0:2, :])
                            elif ec == 3:
                                nc.sync.dma_start(
                                    out=out.ap()[nb * P:(nb + 1) * P,
                                                 1024:2048],
                                    in_=ostage[:, 2:4, :])
                        elif ec == NCH - 1:
                            nc.sync.dma_start(
                                out=out.ap()[nb * P:(nb + 1) * P, :],
                                in_=ostage)
                    return emit

                def queue_ready_op():
                    for cig in range(NCH):
                        if op_queued[cig] or transposed[cig] < HPC:
                            continue
                        if cig > 0 and step[0] < tdone_step[cig] + 1:
                            continue
                        op_queued[cig] = True
                        for nb in range(cig * 4, cig * 4 + 4):
                            for ec in range(NCH):
                                fopq.append((4 * 213, make_op_item(nb, ec)))

                def emit_fill(target_ns, prefer_op):
                    acc = 0
                    while acc < target_ns:
                        if prefer_op and fopq:
                            q = fopq
                        elif favq:
                            q = favq
                        elif fopq:
                            q = fopq
                        else:
                            return
                        cost, fn = q.popleft()
                        fn()
                        acc += cost
                        prefer_op = False

                def mark_transposed(ci):
                    transposed[ci] += 1
                    if transposed[ci] == HPC:
                        tdone_step[ci] = step[0]

                prev = None  # (ci, h, pt, att_h)
                for ci in range(1, NCH):
                    for h in range(HPC):
                        queue_ready_op()
                        if prev is not None:
                            pci, ph, ppt, patt = prev
                            for ib in range(4):
                                def mk(pci=pci, ph=ph, ib=ib, ppt=ppt,
                                       patt=patt):
                                    def em():
                                        av_item(pci, ph, ib, ppt, patt)
                                        if ib == 3:
                                            mark_transposed(pci)
                                    return em
                                favq.append(
                                    ((4 * pci + ib + 1) * 54 + 150, mk()))
                        pt = pt_pool.tile([P, ET, 512], FP16, tag="pt")
                        att_h = att_pool.tile([P, NCH, D], FP16, tag="atth")
                        npairs = 2 * ci + 2
                        for pi in range(npairs):
                            # fill BEFORE the pair: the pair's PSUM bank is
                            # gated by an earlier pair's exp, and the PE is
                            # in-order — fill emitted after a stalled matmul
                            # would be stuck behind it
                            emit_fill(
                                400 if pi == npairs - 1 else 700,
                                prefer_op=(pi == 0))
                            emit_qk_pair(ci, h, pt, pi)
                        prev = (ci, h, pt, att_h)
                        step[0] += 1

                # drain: last tile's A@V with PE-side transposes, then the
                # remaining out-proj chunks
                pci, ph, ppt, patt = prev
                while favq:
                    favq.popleft()[1]()
                for ib in range(4):
                    av_item(pci, ph, ib, ppt, patt)
                transposed[pci] = HPC
                tdone_step[pci] = step[0] - 2
                queue_ready_op()
                while fopq:
                    fopq.popleft()[1]()
                step[0] += 4
                queue_ready_op()
                while fopq:
                    fopq.popleft()[1]()

        rs_pool.release()
        att_pool.release()
        pt_pool.release()
        for _pl in reversed(_longlived):
            _pl.release()

    nc.compile()
    return nc


def make_in_maps(x, Wq, Wkv, Wout):
    x = np.asarray(x, dtype=np.float32)
    Wq = np.asarray(Wq, dtype=np.float32)
    Wkv = np.asarray(Wkv, dtype=np.float32)
    Wout = np.asarray(Wout, dtype=np.float32)
    scale = np.float32(D ** -0.5)

    # [strictly-diagonal causal mask (col >= row) | identity]
    jj = np.arange(P)[:, None]
    ii = np.arange(P)[None, :]
    mask = np.zeros((P, 2, P), dtype=np.float16)
    mask[:, 0, :] = (ii >= jj).astype(np.float16)
    mask[:, 1, :] = (ii == jj).astype(np.float16)

    xT = [np.ascontiguousarray(x[b].T).astype(np.float16) for b in range(B)]
    in_maps = []
    for c in range(NCORES):
        b, hg = divmod(c, 4)
        sl = slice(hg * DC, (hg + 1) * DC)
        in_maps.append({
            "xT": xT[b],
            "wqT": (np.ascontiguousarray(Wq[sl, :].T) * scale).astype(np.float16),
            "wkT": np.ascontiguousarray(Wkv[sl, :].T).astype(np.float16),
            "wvT": np.ascontiguousarray(Wkv[E + sl.start:E + sl.stop, :].T).astype(np.float16),
            "woT": np.ascontiguousarray(Wout[:, sl].T).astype(np.float16),
            "maskin": mask,
        })
    return in_maps


_NC_CACHE = []


def _get_nc():
    if not _NC_CACHE:
        _NC_CACHE.append(build_nc())
    return _NC_CACHE[0]


def _run(in_maps):
    nc = _get_nc()
    return run_bass_kernel_spmd(nc, in_maps, core_ids=list(range(NCORES)))


def kernel(x, Wq, Wkv, Wout):
    in_maps = make_in_maps(x, Wq, Wkv, Wout)
    res = _run(in_maps)
    out = np.zeros((B, N, E), dtype=np.float32)
    for c in range(NCORES):
        out[c // 4] += res.results[c]["out"].astype(np.float32)
    return out


if __name__ == "__main__":
    t0 = time.time()
    _get_nc()
    print(f"build+compile: {time.time() - t0:.1f}s")


# revision 29
# speedup vs baseline: 1.0795x; 1.0006x over previous
"""Trainium2 Bass kernel for nn_BaseAttention (B=2, N=2048, E=2048, H=16, D=128).

Sharding: 8 cores; core c handles batch b=c//4, head-group hg=c%4 (4 heads).
Each core computes q/k/v projections for its heads, causal flash-style
attention, and a partial out-projection (contraction over its 512 head dims).
Host sums the 4 partial outputs per batch (tensor-parallel unshard).

Schedule (v3):
- QK^T and A@V are causally exact at 128-block granularity; only the
  strictly-diagonal 128x128 squares get a triangular mask multiply.
- The four ci=0 attention tiles (which need only chunk 0 of q/k/v) are woven
  into the projection phase, so their exp/spill/transpose chain finishes long
  before the attention phase starts.
- In the attention phase, out-projection matmuls and the previous tile's A@V
  are emitted as PE fill work BEFORE each QK PSUM pair, so the in-order PE
  never parks on the ScalarE exp cadence.
- The last attention row's outputs are transposed on the PE (via identity
  matmul) instead of the DRAM round-trip, removing the final transpose DMA
  latency from the critical path.
- A dummy warmup matmul chain absorbs the PE p-state ramp while the first
  input DMA pieces land; the first projection chunk runs et-outer so the PE
  starts consuming pieces as they arrive.
- Output is written fp16 and summed on host in fp32.
"""

import sys
import time

sys.path.insert(0, "/opt/trn_rl_repo")

from collections import deque

import numpy as np

import concourse.bass as bass
import concourse.mybir as mybir
import concourse.tile as tile
from concourse import bacc
from concourse.bass_utils import run_bass_kernel_spmd

B, N, E, H = 2, 2048, 2048, 16
D = E // H            # 128
HPC = 4               # heads per core
DC = HPC * D          # 512 head dims per core
NCORES = 8
P = 128
NCH = N // 512        # 4 n-chunks of 512
ET = E // P           # 16 e-tiles of 128

F32 = mybir.dt.float32
FP16 = mybir.dt.float16


def build_nc():
    nc = bacc.Bacc("TRN2", target_bir_lowering=False, debug=False,
                   num_devices=NCORES)

    xT = nc.dram_tensor("xT", [E, N], FP16, kind="ExternalInput")
    wqT = nc.dram_tensor("wqT", [E, DC], FP16, kind="ExternalInput")
    wkT = nc.dram_tensor("wkT", [E, DC], FP16, kind="ExternalInput")
    wvT = nc.dram_tensor("wvT", [E, DC], FP16, kind="ExternalInput")
    woT = nc.dram_tensor("woT", [DC, E], FP16, kind="ExternalInput")
    maskin = nc.dram_tensor("maskin", [P, 2, P], FP16, kind="ExternalInput")
    out = nc.dram_tensor("out", [N, E], FP16, kind="ExternalOutput")

    xT_r = xT.ap().rearrange("(eo p) n -> p eo n", p=P)      # [128,16,2048]
    wqT_r = wqT.ap().rearrange("(eo p) d -> p eo d", p=P)    # [128,16,512]
    wkT_r = wkT.ap().rearrange("(eo p) d -> p eo d", p=P)
    wvT_r = wvT.ap().rearrange("(eo p) d -> p eo d", p=P)
    woT_r = woT.ap().rearrange("(t p) e -> p t e", p=P)      # [128,4,2048]

    EXPF = mybir.ActivationFunctionType.Exp

    with tile.TileContext(nc) as tc:
        # ---------------- constants + spill tensors ----------------
        consts = tc.alloc_tile_pool(name="consts", bufs=1)
        _longlived = [consts]
        mask_sb = consts.tile([P, 2, P], FP16)   # [tri(c>=p) | identity]
        warm_sb = consts.tile([P, 512], FP16)
        # prefire the Exp table load so it overlaps the input DMA head
        dummy = consts.tile([1, 8], F32)
        nc.vector.memset(warm_sb, 0.0)
        nc.vector.memset(dummy, 0.0)
        nc.scalar.activation(out=dummy, in_=dummy, func=EXPF)

        # per-core activations, SBUF-resident across the whole kernel
        big = tc.alloc_tile_pool(name="big", bufs=1)
        _longlived.append(big)
        qs = big.tile([P, HPC, N], FP16)                  # q^T, heads stacked
        ks = big.tile([P, HPC, N], FP16)                  # k^T
        v_all = big.tile([P, N // P, HPC, D + 4], FP16)   # [V | 1] per block
        nc.vector.memset(v_all[:, :, :, D:D + 1], 1.0)

        outT_pool = tc.alloc_tile_pool(name="outT", bufs=1)
        _longlived.append(outT_pool)
        outTs = [outT_pool.tile([P, N], FP16, name=f"outT{t}")
                 for t in range(HPC)]
        wo_pool = tc.alloc_tile_pool(name="wo_pool", bufs=1)
        _longlived.append(wo_pool)
        wo_sb = wo_pool.tile([P, HPC, E], FP16)

        pt_pool = tc.alloc_tile_pool(name="pt_pool", bufs=2)
        att_pool = tc.alloc_tile_pool(name="att_pool", bufs=3)
        rs_pool = tc.alloc_tile_pool(name="rs_pool", bufs=8)

        # ---------------- phase 1a: nch-0 projections (et-outer) --------
        wpool = tc.alloc_tile_pool(name="wpool", bufs=1)
        xpool = tc.alloc_tile_pool(name="xpool", bufs=2)
        wq_sb = wpool.tile([P, ET, DC], FP16)
        wk_sb = wpool.tile([P, ET, DC], FP16)
        wv_sb = wpool.tile([P, ET, DC], FP16)
        x_tiles = [None] * NCH

        def load_x(nch):
            t = xpool.tile([P, ET, 512], FP16, tag="xchunk",
                           name=f"x_sb{nch}")
            nc.sync.dma_start(
                out=t, in_=xT_r[:, :, nch * 512:(nch + 1) * 512])
            x_tiles[nch] = t

        with (
            tc.tile_pool(name="warmp", bufs=1, space="PSUM") as warmp,
            tc.tile_pool(name="pj8", bufs=7, space="PSUM") as pj8,
        ):
            # PE warmup: absorb the p-state ramp on dummy matmuls while the
            # first input pieces stream in
            warm_ps = warmp.tile([P, 512], F32, tag="warm")
            for w in range(7):
                nc.tensor.matmul(warm_ps, lhsT=warm_sb[:, 0:P],
                                 rhs=warm_sb, start=(w == 0), stop=(w == 6))

            # small pieces throughout: PE consumption (~0.85us/et) only just
            # trails DMA supply (~0.72us/et), so a late big piece stalls PE
            x0 = xpool.tile([P, ET, 512], FP16, tag="xchunk", name="x_sb0")
            x_tiles[0] = x0
            for a, b in ((0, 1), (1, 2), (2, 4), (4, 6), (6, 8), (8, 10),
                         (10, 12), (12, 14), (14, 16)):
                gs = slice(a, b)
                nc.sync.dma_start(out=wq_sb[:, gs, :], in_=wqT_r[:, gs, :])
                nc.sync.dma_start(out=x0[:, gs, :], in_=xT_r[:, gs, 0:512])
            for g in range(4):
                gs = slice(g * 4, (g + 1) * 4)
                nc.sync.dma_start(out=wk_sb[:, gs, :], in_=wkT_r[:, gs, :])
            load_x(1)
            nc.sync.dma_start(out=mask_sb, in_=maskin.ap())
            HF = ET // 2
            nc.sync.dma_start(out=wv_sb[:, :HF, :], in_=wvT_r[:, :HF, :])
            nc.sync.dma_start(out=wv_sb[:, HF:, :], in_=wvT_r[:, HF:, :])

            for w_sb, dst in ((wq_sb, qs), (wk_sb, ks)):
                pss = [pj8.tile([P, 512], F32, tag="pjps", name=f"pjt{t}")
                       for t in range(HPC)]
                for et in range(ET):
                    for t in range(HPC):
                        nc.tensor.matmul(
                            pss[t],
                            lhsT=w_sb[:, et, t * P:(t + 1) * P],
                            rhs=x0[:, et, :],
                            start=(et == 0), stop=(et == ET - 1),
                        )
                for t in range(HPC):
                    nc.vector.tensor_copy(out=dst[:, t, 0:512], in_=pss[t])
            for nb in range(4):
                ps = pj8.tile([P, 512], F32, tag="pjps")
                for et in range(ET):
                    nc.tensor.matmul(
                        ps,
                        lhsT=x0[:, et, nb * P:(nb + 1) * P],
                        rhs=wv_sb[:, et, :],
                        start=(et == 0), stop=(et == ET - 1),
                    )
                nc.vector.tensor_copy(
                    out=v_all[:, nb, :, :D],
                    in_=ps.rearrange("p (h d) -> p h d", h=HPC))
            load_x(2)
            # wo loads here (DMA slack mid-proj) so the attention-phase DMA
            # queue is free for the attention spills/transposes
            for t in range(HPC):
                nc.sync.dma_start(out=wo_sb[:, t, :], in_=woT_r[:, t, :])

        # ------- phase 1b + 2: proj nch 1-3 (with ci=0 attn tiles woven
        # in), then attention rows 1-3 merged with the out-projection -----
        if True:
            psum_pools = {}
            tri = mask_sb[:, 0, :]
            ident = mask_sb[:, 1, :]

            # ---- attention tile helpers (used for ci=0 during proj and
            # for rows 1..3 in the attention phase) ----
            def emit_qk_pair(ci, h, pt, pi):
                npairs = 2 * ci + 2
                bj0, bj1 = 2 * pi, 2 * pi + 1
                ps = psum_pools["qk"].tile([P, 2, 512], F32, tag="qkps")
                for u, bj in ((0, bj0), (1, bj1)):
                    rr = bj - 4 * ci
                    if rr <= 0:
                        nc.tensor.matmul(
                            ps[:, u, :],
                            lhsT=ks[:, h, bj * P:(bj + 1) * P],
                            rhs=qs[:, h, ci * 512:(ci + 1) * 512],
                            start=True, stop=True,
                        )
                    else:
                        nc.tensor.matmul(
                            ps[:, u, rr * P:],
                            lhsT=ks[:, h, bj * P:(bj + 1) * P],
                            rhs=qs[:, h, ci * 512 + rr * P:(ci + 1) * 512],
                            start=True, stop=True,
                        )
                if pi == npairs - 1:
                    # diagonal pair: only causally-valid columns
                    nc.scalar.activation(
                        out=pt[:, bj0, 256:], in_=ps[:, 0, 256:], func=EXPF)
                    nc.scalar.activation(
                        out=pt[:, bj1, 384:], in_=ps[:, 1, 384:], func=EXPF)
                else:
                    nc.scalar.activation(
                        out=pt[:, bj0:bj0 + 2, :], in_=ps, func=EXPF)
                # triangular mask on strictly-diagonal 128x128 squares
                for u, bj in ((0, bj0), (1, bj1)):
                    rr = bj - 4 * ci
                    if rr >= 0:
                        sq = slice(rr * P, (rr + 1) * P)
                        nc.vector.tensor_mul(
                            out=pt[:, bj, sq], in0=pt[:, bj, sq], in1=tri)

            def av_item(ci, h, ib, pt, att_h):
                gi = 4 * ci + ib
                avp = psum_pools["av"].tile([P, D + 4], F32, tag="avps")
                isl = slice(ib * P, (ib + 1) * P)
                for bj in range(gi + 1):
                    nc.tensor.matmul(
                        avp[:, :D + 1],
                        lhsT=pt[:, bj, isl],
                        rhs=v_all[:, bj, h, :D + 1],
                        start=(bj == 0), stop=(bj == gi),
                    )
                rs = rs_pool.tile([P, 1], F32, tag="rs")
                nc.vector.reciprocal_approx_fast(out=rs, in_=avp[:, D:D + 1])
                nc.vector.tensor_scalar_mul(
                    out=att_h[:, ib, :], in0=avp[:, :D], scalar1=rs)
                # transpose on the PE (identity matmul): no DRAM round
                # trip, so out-proj groups unlock right after the eviction
                tp = psum_pools["av"].tile([P, D], FP16, tag="avps", name="tps")
                nc.tensor.transpose(tp, att_h[:, ib, :], ident)
                nc.vector.tensor_copy(
                    out=outTs[h][:, ci * 512 + ib * P:
                                 ci * 512 + (ib + 1) * P],
                    in_=tp)

            # ---- proj nch 1..3 with ci=0 tiles woven between chains ----
            with (
                tc.tile_pool(name="pj2", bufs=3, space="PSUM") as pj2,
                tc.tile_pool(name="qkw", bufs=1, space="PSUM") as qkw,
                tc.tile_pool(name="avw", bufs=1, space="PSUM") as avw,
            ):
                psum_pools["qk"] = qkw
                psum_pools["av"] = avw
                pts0 = [pt_pool.tile([P, ET, 512], FP16, tag="pt",
                                     name=f"pt0{h}") for h in range(HPC)]
                atts0 = [att_pool.tile([P, NCH, D], FP16, tag="atth",
                                       name=f"att0{h}") for h in range(HPC)]
                # (kind, args): proj chain steps interleaved with ci=0 work
                weave = deque()
                for h in range(HPC):
                    weave.append(("qk", h, 0))
                    weave.append(("qk", h, 1))
                    for ib in range(4):
                        weave.append(("av", h, ib))

                def weave_step(budget):
                    # pop ci=0 attn pieces; each is tiny vs a proj chain
                    n = 0
                    while weave and n < budget:
                        kind, h, idx = weave[0]
                        if kind == "qk":
                            emit_qk_pair(0, h, pts0[h], idx)
                        else:
                            av_item(0, h, idx, pts0[h], atts0[h])
                        weave.popleft()
                        n += 1

                for nch in range(1, NCH):
                    x_sb = x_tiles[nch]
                    nsl = slice(nch * 512, (nch + 1) * 512)
                    if nch == 1:
                        weave_step(2)

                    for w_sb, dst in ((wq_sb, qs), (wk_sb, ks)):
                        for t in range(HPC):
                            ps = pj2.tile([P, 512], F32, tag="pjps")
                            for et in range(ET):
                                nc.tensor.matmul(
                                    ps,
                                    lhsT=w_sb[:, et, t * P:(t + 1) * P],
                                    rhs=x_sb[:, et, :],
                                    start=(et == 0), stop=(et == ET - 1),
                                )
                            if nch == NCH - 1:
                                nc.scalar.copy(out=dst[:, t, nsl], in_=ps)
                            else:
                                nc.vector.tensor_copy(
                                    out=dst[:, t, nsl], in_=ps)
                            weave_step(1)

                    for nb in range(4):
                        ps = pj2.tile([P, 512], F32, tag="pjps")
                        for et in range(ET):
                            nc.tensor.matmul(
                                ps,
                                lhsT=x_sb[:, et, nb * P:(nb + 1) * P],
                                rhs=wv_sb[:, et, :],
                                start=(et == 0), stop=(et == ET - 1),
                            )
                        if nch == NCH - 1:
                            nc.scalar.copy(
                                out=v_all[:, nch * 4 + nb, :, :D],
                                in_=ps.rearrange("p (h d) -> p h d", h=HPC))
                        else:
                            nc.vector.tensor_copy(
                                out=v_all[:, nch * 4 + nb, :, :D],
                                in_=ps.rearrange("p (h d) -> p h d", h=HPC))
                        weave_step(1)

                    if nch + 2 < NCH:
                        load_x(nch + 2)
                weave_step(99)
            xpool.release()
            wpool.release()

            # ---- attention rows 1..3 + out-projection fill ----
            with (
                tc.tile_pool(name="qk_ps", bufs=2, space="PSUM") as qk_ps,
                tc.tile_pool(name="av_ps", bufs=2, space="PSUM") as av_ps,
                tc.tile_pool(name="op_ps", bufs=2, space="PSUM") as op_ps,
                tc.tile_pool(name="op_ev", bufs=3) as op_ev,
            ):
                psum_pools["qk"] = qk_ps
                psum_pools["av"] = av_ps
                favq = deque()     # (cost_ns, emit_fn) A@V of the prev tile
                fopq = deque()     # (cost_ns, emit_fn) out-proj items
                transposed = [HPC, 0, 0, 0]
                tdone_step = [-99, None, None, None]
                op_queued = [False] * NCH
                op_state = {}      # nb -> ostage tile
                evict_flip = [0]
                step = [0]

                def make_op_item(nb, ec):
                    def emit():
                        if ec == 0:
                            op_state[nb] = op_ev.tile(
                                [P, NCH, 512], FP16, tag="opev",
                                name=f"ost{nb}")
                        ostage = op_state[nb]
                        ps = op_ps.tile([P, 512], F32, tag="opps")
                        for t in range(HPC):
                            nc.tensor.matmul(
                                ps,
                                lhsT=outTs[t][:, nb * P:(nb + 1) * P],
                                rhs=wo_sb[:, t, ec * 512:(ec + 1) * 512],
                                start=(t == 0), stop=(t == HPC - 1),
                            )
                        if evict_flip[0] == 0:
                            nc.vector.tensor_copy(out=ostage[:, ec, :],
                                                  in_=ps)
                        else:
                            nc.scalar.copy(out=ostage[:, ec, :], in_=ps)
                        evict_flip[0] ^= 1
                        if nb == 4 * NCH - 1:
                            # very last row-block: per-ec DMAs so the final
                            # transfer trailing the last matmul is small
                            nc.sync.dma_start(
                                out=out.ap()[nb * P:(nb + 1) * P,
                                             ec * 512:(ec + 1) * 512],
                                in_=ostage[:, ec, :])
                        elif nb >= 4 * (NCH - 1):
                            if ec == 1:
                                nc.sync.dma_start(
                                    out=out.ap()[nb * P:(nb + 1) * P,
                                                 0:1024],
                                    in_=ostage[:, 0:2, :])
                            elif ec == 3:
                                nc.sync.dma_start(
                                    out=out.ap()[nb * P:(nb + 1) * P,
                                                 1024:2048],
                                    in_=ostage[:, 2:4, :])
                        elif ec == NCH - 1:
                            nc.sync.dma_start(
                                out=out.ap()[nb * P:(nb + 1) * P, :],
                                in_=ostage)
                    return emit

                def queue_ready_op():
                    for cig in range(NCH):
                        if op_queued[cig] or transposed[cig] < HPC:
                            continue
                        if cig > 0 and step[0] < tdone_step[cig] + 1:
                            continue
                        op_queued[cig] = True
                        for nb in range(cig * 4, cig * 4 + 4):
                            for ec in range(NCH):
                                fopq.append((4 * 213, make_op_item(nb, ec)))

                def emit_fill(target_ns, prefer_op):
                    acc = 0
                    while acc < target_ns:
                        if prefer_op and fopq:
                            q = fopq
                        elif favq:
                            q = favq
                        elif fopq:
                            q = fopq
                        else:
                            return
                        cost, fn = q.popleft()
                        fn()
                        acc += cost
                        prefer_op = False

                def mark_transposed(ci):
                    transposed[ci] += 1
                    if transposed[ci] == HPC:
                        tdone_step[ci] = step[0]

                prev = None  # (ci, h, pt, att_h)
                for ci in range(1, NCH):
                    for h in range(HPC):
                        queue_ready_op()
                        if prev is not None:
                            pci, ph, ppt, patt = prev
                            for ib in range(4):
                                def mk(pci=pci, ph=ph, ib=ib, ppt=ppt,
                                       patt=patt):
                                    def em():
                                        av_item(pci, ph, ib, ppt, patt)
                                        if ib == 3:
                                            mark_transposed(pci)
                                    return em
                                favq.append(
                                    ((4 * pci + ib + 1) * 54 + 150, mk()))
                        pt = pt_pool.tile([P, ET, 512], FP16, tag="pt")
                        att_h = att_pool.tile([P, NCH, D], FP16, tag="atth")
                        npairs = 2 * ci + 2
                        for pi in range(npairs):
                            # fill BEFORE the pair: the pair's PSUM bank is
                            # gated by an earlier pair's exp, and the PE is
                            # in-order — fill emitted after a stalled matmul
                            # would be stuck behind it
                            emit_fill(
                                400 if pi == npairs - 1 else 800,
                                prefer_op=(pi == 0))
                            emit_qk_pair(ci, h, pt, pi)
                        prev = (ci, h, pt, att_h)
                        step[0] += 1

                # drain: last tile's A@V with PE-side transposes, then the
                # remaining out-proj chunks
                pci, ph, ppt, patt = prev
                while favq:
                    favq.popleft()[1]()
                for ib in range(4):
                    av_item(pci, ph, ib, ppt, patt)
                transposed[pci] = HPC
                tdone_step[pci] = step[0] - 2
                queue_ready_op()
                while fopq:
                    fopq.popleft()[1]()
                step[0] += 4
                queue_ready_op()
                while fopq:
                    fopq.popleft()[1]()

        rs_pool.release()
        att_pool.release()
        pt_pool.release()
        for _pl in reversed(_longlived):
            _pl.release()

    nc.compile()
    return nc


def make_in_maps(x, Wq, Wkv, Wout):
    x = np.asarray(x, dtype=np.float32)
    Wq = np.asarray(Wq, dtype=np.float32)
    Wkv = np.asarray(Wkv, dtype=np.float32)
    Wout = np.asarray(Wout, dtype=np.float32)
    scale = np.float32(D ** -0.5)

    # [strictly-diagonal causal mask (col >= row) | identity]
    jj = np.arange(P)[:, None]
    ii = np.arange(P)[None, :]
    mask = np.zeros((P, 2, P), dtype=np.float16)
    mask[:, 0, :] = (ii >= jj).astype(np.float16)
    mask[:, 1, :] = (ii == jj).astype(np.float16)

    xT = [np.ascontiguousarray(x[b].T).astype(np.float16) for b in range(B)]
    in_maps = []
    for c in range(NCORES):
        b, hg = divmod(c, 4)
        sl = slice(hg * DC, (hg + 1) * DC)
        in_maps.append({
            "xT": xT[b],
            "wqT": (np.ascontiguousarray(Wq[sl, :].T) * scale).astype(np.float16),
            "wkT": np.ascontiguousarray(Wkv[sl, :].T).astype(np.float16),
            "wvT": np.ascontiguousarray(Wkv[E + sl.start:E + sl.stop, :].T).astype(np.float16),
            "woT": np.ascontiguousarray(Wout[:, sl].T).astype(np.float16),
            "maskin": mask,
        })
    return in_maps


_NC_CACHE = []


def _get_nc():
    if not _NC_CACHE:
        _NC_CACHE.append(build_nc())
    return _NC_CACHE[0]


def _run(in_maps):
    nc = _get_nc()
    return run_bass_kernel_spmd(nc, in_maps, core_ids=list(range(NCORES)))


def kernel(x, Wq, Wkv, Wout):
    in_maps = make_in_maps(x, Wq, Wkv, Wout)
    res = _run(in_maps)
    out = np.zeros((B, N, E), dtype=np.float32)
    for c in range(NCORES):
        out[c // 4] += res.results[c]["out"].astype(np.float32)
    return out


if __name__ == "__main__":
    t0 = time.time()
    _get_nc()
    print(f"build+compile: {time.time() - t0:.1f}s")


# revision 31
# speedup vs baseline: 1.0798x; 1.0002x over previous
"""Trainium2 Bass kernel for nn_BaseAttention (B=2, N=2048, E=2048, H=16, D=128).

Sharding: 8 cores; core c handles batch b=c//4, head-group hg=c%4 (4 heads).
Each core computes q/k/v projections for its heads, causal flash-style
attention, and a partial out-projection (contraction over its 512 head dims).
Host sums the 4 partial outputs per batch (tensor-parallel unshard).

Schedule (v3):
- QK^T and A@V are causally exact at 128-block granularity; only the
  strictly-diagonal 128x128 squares get a triangular mask multiply.
- The four ci=0 attention tiles (which need only chunk 0 of q/k/v) are woven
  into the projection phase, so their exp/spill/transpose chain finishes long
  before the attention phase starts.
- In the attention phase, out-projection matmuls and the previous tile's A@V
  are emitted as PE fill work BEFORE each QK PSUM pair, so the in-order PE
  never parks on the ScalarE exp cadence.
- The last attention row's outputs are transposed on the PE (via identity
  matmul) instead of the DRAM round-trip, removing the final transpose DMA
  latency from the critical path.
- A dummy warmup matmul chain absorbs the PE p-state ramp while the first
  input DMA pieces land; the first projection chunk runs et-outer so the PE
  starts consuming pieces as they arrive.
- Output is written fp16 and summed on host in fp32.
"""

import sys
import time

sys.path.insert(0, "/opt/trn_rl_repo")

from collections import deque

import numpy as np

import concourse.bass as bass
import concourse.mybir as mybir
import concourse.tile as tile
from concourse import bacc
from concourse.bass_utils import run_bass_kernel_spmd

B, N, E, H = 2, 2048, 2048, 16
D = E // H            # 128
HPC = 4               # heads per core
DC = HPC * D          # 512 head dims per core
NCORES = 8
P = 128
NCH = N // 512        # 4 n-chunks of 512
ET = E // P           # 16 e-tiles of 128

F32 = mybir.dt.float32
FP16 = mybir.dt.float16


def build_nc():
    nc = bacc.Bacc("TRN2", target_bir_lowering=False, debug=False,
                   num_devices=NCORES)

    xT = nc.dram_tensor("xT", [E, N], FP16, kind="ExternalInput")
    wqT = nc.dram_tensor("wqT", [E, DC], FP16, kind="ExternalInput")
    wkT = nc.dram_tensor("wkT", [E, DC], FP16, kind="ExternalInput")
    wvT = nc.dram_tensor("wvT", [E, DC], FP16, kind="ExternalInput")
    woT = nc.dram_tensor("woT", [DC, E], FP16, kind="ExternalInput")
    maskin = nc.dram_tensor("maskin", [P, 2, P], FP16, kind="ExternalInput")
    out = nc.dram_tensor("out", [N, E], FP16, kind="ExternalOutput")

    xT_r = xT.ap().rearrange("(eo p) n -> p eo n", p=P)      # [128,16,2048]
    wqT_r = wqT.ap().rearrange("(eo p) d -> p eo d", p=P)    # [128,16,512]
    wkT_r = wkT.ap().rearrange("(eo p) d -> p eo d", p=P)
    wvT_r = wvT.ap().rearrange("(eo p) d -> p eo d", p=P)
    woT_r = woT.ap().rearrange("(t p) e -> p t e", p=P)      # [128,4,2048]

    EXPF = mybir.ActivationFunctionType.Exp

    with tile.TileContext(nc) as tc:
        # ---------------- constants + spill tensors ----------------
        consts = tc.alloc_tile_pool(name="consts", bufs=1)
        _longlived = [consts]
        mask_sb = consts.tile([P, 2, P], FP16)   # [tri(c>=p) | identity]
        warm_sb = consts.tile([P, 512], FP16)
        # prefire the Exp table load so it overlaps the input DMA head
        dummy = consts.tile([1, 8], F32)
        nc.gpsimd.memset(warm_sb, 0.0)
        nc.vector.memset(dummy, 0.0)
        nc.scalar.activation(out=dummy, in_=dummy, func=EXPF)

        # per-core activations, SBUF-resident across the whole kernel
        big = tc.alloc_tile_pool(name="big", bufs=1)
        _longlived.append(big)
        qs = big.tile([P, HPC, N], FP16)                  # q^T, heads stacked
        ks = big.tile([P, HPC, N], FP16)                  # k^T
        v_all = big.tile([P, N // P, HPC, D + 4], FP16)   # [V | 1] per block
        nc.vector.memset(v_all[:, :, :, D:D + 1], 1.0)

        outT_pool = tc.alloc_tile_pool(name="outT", bufs=1)
        _longlived.append(outT_pool)
        outTs = [outT_pool.tile([P, N], FP16, name=f"outT{t}")
                 for t in range(HPC)]
        wo_pool = tc.alloc_tile_pool(name="wo_pool", bufs=1)
        _longlived.append(wo_pool)
        wo_sb = wo_pool.tile([P, HPC, E], FP16)

        pt_pool = tc.alloc_tile_pool(name="pt_pool", bufs=2)
        att_pool = tc.alloc_tile_pool(name="att_pool", bufs=3)
        rs_pool = tc.alloc_tile_pool(name="rs_pool", bufs=8)

        # ---------------- phase 1a: nch-0 projections (et-outer) --------
        wpool = tc.alloc_tile_pool(name="wpool", bufs=1)
        xpool = tc.alloc_tile_pool(name="xpool", bufs=2)
        wq_sb = wpool.tile([P, ET, DC], FP16)
        wk_sb = wpool.tile([P, ET, DC], FP16)
        wv_sb = wpool.tile([P, ET, DC], FP16)
        x_tiles = [None] * NCH

        def load_x(nch):
            t = xpool.tile([P, ET, 512], FP16, tag="xchunk",
                           name=f"x_sb{nch}")
            nc.sync.dma_start(
                out=t, in_=xT_r[:, :, nch * 512:(nch + 1) * 512])
            x_tiles[nch] = t

        with (
            tc.tile_pool(name="warmp", bufs=1, space="PSUM") as warmp,
            tc.tile_pool(name="pj8", bufs=6, space="PSUM") as pj8,
        ):
            # PE warmup: absorb the p-state ramp on dummy matmuls while the
            # first input pieces stream in
            warm_ps = warmp.tile([P, 512], F32, tag="warm")
            for w in range(7):
                nc.tensor.matmul(warm_ps, lhsT=warm_sb[:, 0:P],
                                 rhs=warm_sb, start=(w == 0), stop=(w == 6))

            # small pieces throughout: PE consumption (~0.85us/et) only just
            # trails DMA supply (~0.72us/et), so a late big piece stalls PE
            x0 = xpool.tile([P, ET, 512], FP16, tag="xchunk", name="x_sb0")
            x_tiles[0] = x0
            for a, b in ((0, 1), (1, 2), (2, 4), (4, 6), (6, 8), (8, 10),
                         (10, 12), (12, 14), (14, 16)):
                gs = slice(a, b)
                nc.sync.dma_start(out=wq_sb[:, gs, :], in_=wqT_r[:, gs, :])
                nc.sync.dma_start(out=x0[:, gs, :], in_=xT_r[:, gs, 0:512])
            for g in range(4):
                gs = slice(g * 4, (g + 1) * 4)
                nc.sync.dma_start(out=wk_sb[:, gs, :], in_=wkT_r[:, gs, :])
            load_x(1)
            nc.sync.dma_start(out=mask_sb, in_=maskin.ap())
            HF = ET // 2
            nc.sync.dma_start(out=wv_sb[:, :HF, :], in_=wvT_r[:, :HF, :])
            nc.sync.dma_start(out=wv_sb[:, HF:, :], in_=wvT_r[:, HF:, :])

            for w_sb, dst in ((wq_sb, qs), (wk_sb, ks)):
                pss = [pj8.tile([P, 512], F32, tag="pjps", name=f"pjt{t}")
                       for t in range(HPC)]
                for et in range(ET):
                    for t in range(HPC):
                        nc.tensor.matmul(
                            pss[t],
                            lhsT=w_sb[:, et, t * P:(t + 1) * P],
                            rhs=x0[:, et, :],
                            start=(et == 0), stop=(et == ET - 1),
                        )
                for t in range(HPC):
                    nc.vector.tensor_copy(out=dst[:, t, 0:512], in_=pss[t])
            for nb in range(4):
                ps = pj8.tile([P, 512], F32, tag="pjps")
                for et in range(ET):
                    nc.tensor.matmul(
                        ps,
                        lhsT=x0[:, et, nb * P:(nb + 1) * P],
                        rhs=wv_sb[:, et, :],
                        start=(et == 0), stop=(et == ET - 1),
                    )
                nc.vector.tensor_copy(
                    out=v_all[:, nb, :, :D],
                    in_=ps.rearrange("p (h d) -> p h d", h=HPC))
                if nb == 2:
                    # first nch-1 q chain here: its PE work hides the last
                    # v eviction that gates the next pool's coarse sems
                    ps_q1 = pj8.tile([P, 512], F32, tag="pjps",
                                     name="psq1")
                    for et in range(ET):
                        nc.tensor.matmul(
                            ps_q1,
                            lhsT=wq_sb[:, et, 0:P],
                            rhs=x_tiles[1][:, et, :],
                            start=(et == 0), stop=(et == ET - 1),
                        )
                    nc.vector.tensor_copy(out=qs[:, 0, 512:1024],
                                          in_=ps_q1)
            load_x(2)
            # wo loads here (DMA slack mid-proj) so the attention-phase DMA
            # queue is free for the attention spills/transposes
            for t in range(HPC):
                nc.sync.dma_start(out=wo_sb[:, t, :], in_=woT_r[:, t, :])

        # ------- phase 1b + 2: proj nch 1-3 (with ci=0 attn tiles woven
        # in), then attention rows 1-3 merged with the out-projection -----
        if True:
            psum_pools = {}
            tri = mask_sb[:, 0, :]
            ident = mask_sb[:, 1, :]

            # ---- attention tile helpers (used for ci=0 during proj and
            # for rows 1..3 in the attention phase) ----
            def emit_qk_pair(ci, h, pt, pi):
                npairs = 2 * ci + 2
                bj0, bj1 = 2 * pi, 2 * pi + 1
                ps = psum_pools["qk"].tile([P, 2, 512], F32, tag="qkps")
                for u, bj in ((0, bj0), (1, bj1)):
                    rr = bj - 4 * ci
                    if rr <= 0:
                        nc.tensor.matmul(
                            ps[:, u, :],
                            lhsT=ks[:, h, bj * P:(bj + 1) * P],
                            rhs=qs[:, h, ci * 512:(ci + 1) * 512],
                            start=True, stop=True,
                        )
                    else:
                        nc.tensor.matmul(
                            ps[:, u, rr * P:],
                            lhsT=ks[:, h, bj * P:(bj + 1) * P],
                            rhs=qs[:, h, ci * 512 + rr * P:(ci + 1) * 512],
                            start=True, stop=True,
                        )
                if pi == npairs - 1:
                    # diagonal pair: only causally-valid columns
                    nc.scalar.activation(
                        out=pt[:, bj0, 256:], in_=ps[:, 0, 256:], func=EXPF)
                    nc.scalar.activation(
                        out=pt[:, bj1, 384:], in_=ps[:, 1, 384:], func=EXPF)
                else:
                    nc.scalar.activation(
                        out=pt[:, bj0:bj0 + 2, :], in_=ps, func=EXPF)
                # triangular mask on strictly-diagonal 128x128 squares
                for u, bj in ((0, bj0), (1, bj1)):
                    rr = bj - 4 * ci
                    if rr >= 0:
                        sq = slice(rr * P, (rr + 1) * P)
                        nc.vector.tensor_mul(
                            out=pt[:, bj, sq], in0=pt[:, bj, sq], in1=tri)

            def av_item(ci, h, ib, pt, att_h):
                gi = 4 * ci + ib
                avp = psum_pools["av"].tile([P, D + 4], F32, tag="avps")
                isl = slice(ib * P, (ib + 1) * P)
                for bj in range(gi + 1):
                    nc.tensor.matmul(
                        avp[:, :D + 1],
                        lhsT=pt[:, bj, isl],
                        rhs=v_all[:, bj, h, :D + 1],
                        start=(bj == 0), stop=(bj == gi),
                    )
                rs = rs_pool.tile([P, 1], F32, tag="rs")
                nc.vector.reciprocal_approx_fast(out=rs, in_=avp[:, D:D + 1])
                nc.vector.tensor_scalar_mul(
                    out=att_h[:, ib, :], in0=avp[:, :D], scalar1=rs)
                # transpose on the PE (identity matmul): no DRAM round
                # trip, so out-proj groups unlock right after the eviction
                tp = psum_pools["av"].tile([P, D], FP16, tag="avps", name="tps")
                nc.tensor.transpose(tp, att_h[:, ib, :], ident)
                nc.vector.tensor_copy(
                    out=outTs[h][:, ci * 512 + ib * P:
                                 ci * 512 + (ib + 1) * P],
                    in_=tp)

            # ---- proj nch 1..3 with ci=0 tiles woven between chains ----
            with (
                tc.tile_pool(name="pj2", bufs=3, space="PSUM") as pj2,
                tc.tile_pool(name="qkw", bufs=1, space="PSUM") as qkw,
                tc.tile_pool(name="avw", bufs=1, space="PSUM") as avw,
            ):
                psum_pools["qk"] = qkw
                psum_pools["av"] = avw
                pts0 = [pt_pool.tile([P, ET, 512], FP16, tag="pt",
                                     name=f"pt0{h}") for h in range(HPC)]
                atts0 = [att_pool.tile([P, NCH, D], FP16, tag="atth",
                                       name=f"att0{h}") for h in range(HPC)]
                # (kind, args): proj chain steps interleaved with ci=0 work
                weave = deque()
                for h in range(HPC):
                    weave.append(("qk", h, 0))
                    weave.append(("qk", h, 1))
                    for ib in range(4):
                        weave.append(("av", h, ib))

                def weave_step(budget):
                    # pop ci=0 attn pieces; each is tiny vs a proj chain
                    n = 0
                    while weave and n < budget:
                        kind, h, idx = weave[0]
                        if kind == "qk":
                            emit_qk_pair(0, h, pts0[h], idx)
                        else:
                            av_item(0, h, idx, pts0[h], atts0[h])
                        weave.popleft()
                        n += 1

                for nch in range(1, NCH):
                    x_sb = x_tiles[nch]
                    nsl = slice(nch * 512, (nch + 1) * 512)

                    for w_sb, dst in ((wq_sb, qs), (wk_sb, ks)):
                        for t in range(HPC):
                            if nch == 1 and w_sb is wq_sb and t == 0:
                                continue
                            ps = pj2.tile([P, 512], F32, tag="pjps")
                            for et in range(ET):
                                nc.tensor.matmul(
                                    ps,
                                    lhsT=w_sb[:, et, t * P:(t + 1) * P],
                                    rhs=x_sb[:, et, :],
                                    start=(et == 0), stop=(et == ET - 1),
                                )
                            if nch == NCH - 1:
                                nc.scalar.copy(out=dst[:, t, nsl], in_=ps)
                            else:
                                nc.vector.tensor_copy(
                                    out=dst[:, t, nsl], in_=ps)
                            weave_step(1)

                    for nb in range(4):
                        ps = pj2.tile([P, 512], F32, tag="pjps")
                        for et in range(ET):
                            nc.tensor.matmul(
                                ps,
                                lhsT=x_sb[:, et, nb * P:(nb + 1) * P],
                                rhs=wv_sb[:, et, :],
                                start=(et == 0), stop=(et == ET - 1),
                            )
                        if nch == NCH - 1:
                            nc.scalar.copy(
                                out=v_all[:, nch * 4 + nb, :, :D],
                                in_=ps.rearrange("p (h d) -> p h d", h=HPC))
                        else:
                            nc.vector.tensor_copy(
                                out=v_all[:, nch * 4 + nb, :, :D],
                                in_=ps.rearrange("p (h d) -> p h d", h=HPC))
                        weave_step(1)

                    if nch + 2 < NCH:
                        load_x(nch + 2)
                weave_step(99)
            xpool.release()
            wpool.release()

            # ---- attention rows 1..3 + out-projection fill ----
            with (
                tc.tile_pool(name="qk_ps", bufs=2, space="PSUM") as qk_ps,
                tc.tile_pool(name="av_ps", bufs=2, space="PSUM") as av_ps,
                tc.tile_pool(name="op_ps", bufs=2, space="PSUM") as op_ps,
                tc.tile_pool(name="op_ev", bufs=3) as op_ev,
            ):
                psum_pools["qk"] = qk_ps
                psum_pools["av"] = av_ps
                favq = deque()     # (cost_ns, emit_fn) A@V of the prev tile
                fopq = deque()     # (cost_ns, emit_fn) out-proj items
                transposed = [HPC, 0, 0, 0]
                tdone_step = [-99, None, None, None]
                op_queued = [False] * NCH
                op_state = {}      # nb -> ostage tile
                evict_flip = [0]
                step = [0]

                def make_op_item(nb, ec):
                    def emit():
                        if ec == 0:
                            op_state[nb] = op_ev.tile(
                                [P, NCH, 512], FP16, tag="opev",
                                name=f"ost{nb}")
                        ostage = op_state[nb]
                        ps = op_ps.tile([P, 512], F32, tag="opps")
                        for t in range(HPC):
                            nc.tensor.matmul(
                                ps,
                                lhsT=outTs[t][:, nb * P:(nb + 1) * P],
                                rhs=wo_sb[:, t, ec * 512:(ec + 1) * 512],
                                start=(t == 0), stop=(t == HPC - 1),
                            )
                        if evict_flip[0] == 0:
                            nc.vector.tensor_copy(out=ostage[:, ec, :],
                                                  in_=ps)
                        else:
                            nc.scalar.copy(out=ostage[:, ec, :], in_=ps)
                        evict_flip[0] ^= 1
                        if nb == 4 * NCH - 1:
                            # very last row-block: per-ec DMAs so the final
                            # transfer trailing the last matmul is small
                            nc.sync.dma_start(
                                out=out.ap()[nb * P:(nb + 1) * P,
                                             ec * 512:(ec + 1) * 512],
                                in_=ostage[:, ec, :])
                        elif nb >= 4 * (NCH - 1):
                            if ec == 1:
                                nc.sync.dma_start(
                                    out=out.ap()[nb * P:(nb + 1) * P,
                                                 0:1024],
                                    in_=ostage[:, 0:2, :])
                            elif ec == 3:
                                nc.sync.dma_start(
                                    out=out.ap()[nb * P:(nb + 1) * P,
                                                 1024:2048],
                                    in_=ostage[:, 2:4, :])
                        elif ec == NCH - 1:
                            nc.sync.dma_start(
                                out=out.ap()[nb * P:(nb + 1) * P, :],
                                in_=ostage)
                    return emit

                def queue_ready_op():
                    for cig in range(NCH):
                        if op_queued[cig] or transposed[cig] < HPC:
                            continue
                        if cig > 0 and step[0] < tdone_step[cig] + 1:
                            continue
                        op_queued[cig] = True
                        for nb in range(cig * 4, cig * 4 + 4):
                            for ec in range(NCH):
                                fopq.append((4 * 213, make_op_item(nb, ec)))

                def emit_fill(target_ns, prefer_op):
                    acc = 0
                    while acc < target_ns:
                        if prefer_op and fopq:
                            q = fopq
                        elif favq:
                            q = favq
                        elif fopq:
                            q = fopq
                        else:
                            return
                        cost, fn = q.popleft()
                        fn()
                        acc += cost
                        prefer_op = False

                def mark_transposed(ci):
                    transposed[ci] += 1
                    if transposed[ci] == HPC:
                        tdone_step[ci] = step[0]

                prev = None  # (ci, h, pt, att_h)
                for ci in range(1, NCH):
                    for h in range(HPC):
                        queue_ready_op()
                        if prev is not None:
                            pci, ph, ppt, patt = prev
                            for ib in range(4):
                                def mk(pci=pci, ph=ph, ib=ib, ppt=ppt,
                                       patt=patt):
                                    def em():
                                        av_item(pci, ph, ib, ppt, patt)
                                        if ib == 3:
                                            mark_transposed(pci)
                                    return em
                                favq.append(
                                    ((4 * pci + ib + 1) * 54 + 150, mk()))
                        pt = pt_pool.tile([P, ET, 512], FP16, tag="pt")
                        att_h = att_pool.tile([P, NCH, D], FP16, tag="atth")
                        npairs = 2 * ci + 2
                        for pi in range(npairs):
                            # fill BEFORE the pair: the pair's PSUM bank is
                            # gated by an earlier pair's exp, and the PE is
                            # in-order — fill emitted after a stalled matmul
                            # would be stuck behind it
                            emit_fill(
                                400 if pi == npairs - 1 else 800,
                                prefer_op=(pi == 0))
                            emit_qk_pair(ci, h, pt, pi)
                        prev = (ci, h, pt, att_h)
                        step[0] += 1

                # drain: last tile's A@V with PE-side transposes, then the
                # remaining out-proj chunks
                pci, ph, ppt, patt = prev
                while favq:
                    favq.popleft()[1]()
                for ib in range(4):
                    av_item(pci, ph, ib, ppt, patt)
                transposed[pci] = HPC
                tdone_step[pci] = step[0] - 2
                queue_ready_op()
                while fopq:
                    fopq.popleft()[1]()
                step[0] += 4
                queue_ready_op()
                while fopq:
                    fopq.popleft()[1]()

        rs_pool.release()
        att_pool.release()
        pt_pool.release()
        for _pl in reversed(_longlived):
            _pl.release()

    nc.compile()
    return nc


def make_in_maps(x, Wq, Wkv, Wout):
    x = np.asarray(x, dtype=np.float32)
    Wq = np.asarray(Wq, dtype=np.float32)
    Wkv = np.asarray(Wkv, dtype=np.float32)
    Wout = np.asarray(Wout, dtype=np.float32)
    scale = np.float32(D ** -0.5)

    # [strictly-diagonal causal mask (col >= row) | identity]
    jj = np.arange(P)[:, None]
    ii = np.arange(P)[None, :]
    mask = np.zeros((P, 2, P), dtype=np.float16)
    mask[:, 0, :] = (ii >= jj).astype(np.float16)
    mask[:, 1, :] = (ii == jj).astype(np.float16)

    xT = [np.ascontiguousarray(x[b].T).astype(np.float16) for b in range(B)]
    in_maps = []
    for c in range(NCORES):
        b, hg = divmod(c, 4)
        sl = slice(hg * DC, (hg + 1) * DC)
        in_maps.append({
            "xT": xT[b],
            "wqT": (np.ascontiguousarray(Wq[sl, :].T) * scale).astype(np.float16),
            "wkT": np.ascontiguousarray(Wkv[sl, :].T).astype(np.float16),
            "wvT": np.ascontiguousarray(Wkv[E + sl.start:E + sl.stop, :].T).astype(np.float16),
            "woT": np.ascontiguousarray(Wout[:, sl].T).astype(np.float16),
            "maskin": mask,
        })
    return in_maps


_NC_CACHE = []


def _get_nc():
    if not _NC_CACHE:
        _NC_CACHE.append(build_nc())
    return _NC_CACHE[0]


def _run(in_maps):
    nc = _get_nc()
    return run_bass_kernel_spmd(nc, in_maps, core_ids=list(range(NCORES)))


def kernel(x, Wq, Wkv, Wout):
    in_maps = make_in_maps(x, Wq, Wkv, Wout)
    res = _run(in_maps)
    out = np.zeros((B, N, E), dtype=np.float32)
    for c in range(NCORES):
        out[c // 4] += res.results[c]["out"].astype(np.float32)
    return out


if __name__ == "__main__":
    t0 = time.time()
    _get_nc()
    print(f"build+compile: {time.time() - t0:.1f}s")


# revision 33
# speedup vs baseline: 1.0923x; 1.0116x over previous
"""Trainium2 Bass kernel for nn_BaseAttention (B=2, N=2048, E=2048, H=16, D=128).

Sharding: 8 cores; core c handles batch b=c//4, head-group hg=c%4 (4 heads).
Each core computes q/k/v projections for its heads, causal flash-style
attention, and a partial out-projection (contraction over its 512 head dims).
Host sums the 4 partial outputs per batch (tensor-parallel unshard).

Schedule (v3):
- QK^T and A@V are causally exact at 128-block granularity; only the
  strictly-diagonal 128x128 squares get a triangular mask multiply.
- The four ci=0 attention tiles (which need only chunk 0 of q/k/v) are woven
  into the projection phase, so their exp/spill/transpose chain finishes long
  before the attention phase starts.
- In the attention phase, out-projection matmuls and the previous tile's A@V
  are emitted as PE fill work BEFORE each QK PSUM pair, so the in-order PE
  never parks on the ScalarE exp cadence.
- The last attention row's outputs are transposed on the PE (via identity
  matmul) instead of the DRAM round-trip, removing the final transpose DMA
  latency from the critical path.
- A dummy warmup matmul chain absorbs the PE p-state ramp while the first
  input DMA pieces land; the first projection chunk runs et-outer so the PE
  starts consuming pieces as they arrive.
- Output is written fp16 and summed on host in fp32.
"""

import sys
import time

sys.path.insert(0, "/opt/trn_rl_repo")

from collections import deque

import numpy as np

import concourse.bass as bass
import concourse.mybir as mybir
import concourse.tile as tile
from concourse import bacc
from concourse.bass_utils import run_bass_kernel_spmd

B, N, E, H = 2, 2048, 2048, 16
D = E // H            # 128
HPC = 4               # heads per core
DC = HPC * D          # 512 head dims per core
NCORES = 8
P = 128
NCH = N // 512        # 4 n-chunks of 512
ET = E // P           # 16 e-tiles of 128

F32 = mybir.dt.float32
FP16 = mybir.dt.float16


def build_nc():
    nc = bacc.Bacc("TRN2", target_bir_lowering=False, debug=False,
                   num_devices=NCORES)

    xT = nc.dram_tensor("xT", [E, N], FP16, kind="ExternalInput")
    wqT = nc.dram_tensor("wqT", [E, DC], FP16, kind="ExternalInput")
    wkT = nc.dram_tensor("wkT", [E, DC], FP16, kind="ExternalInput")
    wvT = nc.dram_tensor("wvT", [E, DC], FP16, kind="ExternalInput")
    woT = nc.dram_tensor("woT", [DC, E], FP16, kind="ExternalInput")
    maskin = nc.dram_tensor("maskin", [P, 2, P], FP16, kind="ExternalInput")
    out = nc.dram_tensor("out", [N, E], FP16, kind="ExternalOutput")

    xT_r = xT.ap().rearrange("(eo p) n -> p eo n", p=P)      # [128,16,2048]
    wqT_r = wqT.ap().rearrange("(eo p) d -> p eo d", p=P)    # [128,16,512]
    wkT_r = wkT.ap().rearrange("(eo p) d -> p eo d", p=P)
    wvT_r = wvT.ap().rearrange("(eo p) d -> p eo d", p=P)
    woT_r = woT.ap().rearrange("(t p) e -> p t e", p=P)      # [128,4,2048]

    EXPF = mybir.ActivationFunctionType.Exp

    with tile.TileContext(nc) as tc:
        # ---------------- constants + spill tensors ----------------
        consts = tc.alloc_tile_pool(name="consts", bufs=1)
        _longlived = [consts]
        mask_sb = consts.tile([P, 2, P], FP16)   # [tri(c>=p) | identity]
        warm_sb = consts.tile([P, 512], FP16)
        # prefire the Exp table load so it overlaps the input DMA head
        dummy = consts.tile([1, 8], F32)
        nc.gpsimd.memset(warm_sb, 0.0)
        nc.vector.memset(dummy, 0.0)
        nc.scalar.activation(out=dummy, in_=dummy, func=EXPF)

        dram = tc.alloc_tile_pool(name="dram", bufs=1, space="DRAM")
        _longlived.append(dram)
        attd = dram.tile([HPC, N, D], FP16)          # normalized attn out

        # per-core activations, SBUF-resident across the whole kernel
        big = tc.alloc_tile_pool(name="big", bufs=1)
        _longlived.append(big)
        qs = big.tile([P, HPC, N], FP16)                  # q^T, heads stacked
        ks = big.tile([P, HPC, N], FP16)                  # k^T
        v_all = big.tile([P, N // P, HPC, D + 4], FP16)   # [V | 1] per block
        nc.vector.memset(v_all[:, :, :, D:D + 1], 1.0)

        outT_pool = tc.alloc_tile_pool(name="outT", bufs=1)
        _longlived.append(outT_pool)
        outTs = [outT_pool.tile([P, N], FP16, name=f"outT{t}")
                 for t in range(HPC)]
        wo_pool = tc.alloc_tile_pool(name="wo_pool", bufs=1)
        _longlived.append(wo_pool)
        wo_sb = wo_pool.tile([P, HPC, E], FP16)

        pt_pool = tc.alloc_tile_pool(name="pt_pool", bufs=2)
        att_pool = tc.alloc_tile_pool(name="att_pool", bufs=3)
        rs_pool = tc.alloc_tile_pool(name="rs_pool", bufs=8)

        # ---------------- phase 1a: nch-0 projections (et-outer) --------
        wpool = tc.alloc_tile_pool(name="wpool", bufs=1)
        xpool = tc.alloc_tile_pool(name="xpool", bufs=2)
        wq_sb = wpool.tile([P, ET, DC], FP16)
        wk_sb = wpool.tile([P, ET, DC], FP16)
        wv_sb = wpool.tile([P, ET, DC], FP16)
        x_tiles = [None] * NCH

        def load_x(nch):
            t = xpool.tile([P, ET, 512], FP16, tag="xchunk",
                           name=f"x_sb{nch}")
            nc.sync.dma_start(
                out=t, in_=xT_r[:, :, nch * 512:(nch + 1) * 512])
            x_tiles[nch] = t

        with (
            tc.tile_pool(name="warmp", bufs=1, space="PSUM") as warmp,
            tc.tile_pool(name="pj8", bufs=6, space="PSUM") as pj8,
        ):
            # PE warmup: absorb the p-state ramp on dummy matmuls while the
            # first input pieces stream in
            warm_ps = warmp.tile([P, 512], F32, tag="warm")
            for w in range(7):
                nc.tensor.matmul(warm_ps, lhsT=warm_sb[:, 0:P],
                                 rhs=warm_sb, start=(w == 0), stop=(w == 6))

            # small pieces throughout: PE consumption (~0.85us/et) only just
            # trails DMA supply (~0.72us/et), so a late big piece stalls PE
            x0 = xpool.tile([P, ET, 512], FP16, tag="xchunk", name="x_sb0")
            x_tiles[0] = x0
            for a, b in ((0, 1), (1, 2), (2, 4), (4, 6), (6, 8), (8, 10),
                         (10, 12), (12, 14), (14, 16)):
                gs = slice(a, b)
                nc.sync.dma_start(out=wq_sb[:, gs, :], in_=wqT_r[:, gs, :])
                nc.sync.dma_start(out=x0[:, gs, :], in_=xT_r[:, gs, 0:512])
            for g in range(4):
                gs = slice(g * 4, (g + 1) * 4)
                nc.sync.dma_start(out=wk_sb[:, gs, :], in_=wkT_r[:, gs, :])
            load_x(1)
            nc.sync.dma_start(out=mask_sb, in_=maskin.ap())
            HF = ET // 2
            nc.sync.dma_start(out=wv_sb[:, :HF, :], in_=wvT_r[:, :HF, :])
            nc.sync.dma_start(out=wv_sb[:, HF:, :], in_=wvT_r[:, HF:, :])

            for w_sb, dst in ((wq_sb, qs), (wk_sb, ks)):
                pss = [pj8.tile([P, 512], F32, tag="pjps", name=f"pjt{t}")
                       for t in range(HPC)]
                for et in range(ET):
                    for t in range(HPC):
                        nc.tensor.matmul(
                            pss[t],
                            lhsT=w_sb[:, et, t * P:(t + 1) * P],
                            rhs=x0[:, et, :],
                            start=(et == 0), stop=(et == ET - 1),
                        )
                for t in range(HPC):
                    nc.vector.tensor_copy(out=dst[:, t, 0:512], in_=pss[t])
            for nb in range(4):
                ps = pj8.tile([P, 512], F32, tag="pjps")
                for et in range(ET):
                    nc.tensor.matmul(
                        ps,
                        lhsT=x0[:, et, nb * P:(nb + 1) * P],
                        rhs=wv_sb[:, et, :],
                        start=(et == 0), stop=(et == ET - 1),
                    )
                nc.vector.tensor_copy(
                    out=v_all[:, nb, :, :D],
                    in_=ps.rearrange("p (h d) -> p h d", h=HPC))
                if nb == 2:
                    # first nch-1 q chain here: its PE work hides the last
                    # v eviction that gates the next pool's coarse sems
                    ps_q1 = pj8.tile([P, 512], F32, tag="pjps",
                                     name="psq1")
                    for et in range(ET):
                        nc.tensor.matmul(
                            ps_q1,
                            lhsT=wq_sb[:, et, 0:P],
                            rhs=x_tiles[1][:, et, :],
                            start=(et == 0), stop=(et == ET - 1),
                        )
                    nc.vector.tensor_copy(out=qs[:, 0, 512:1024],
                                          in_=ps_q1)
            load_x(2)
            # wo loads here (DMA slack mid-proj) so the attention-phase DMA
            # queue is free for the attention spills/transposes
            for t in range(HPC):
                nc.sync.dma_start(out=wo_sb[:, t, :], in_=woT_r[:, t, :])

        # ------- phase 1b + 2: proj nch 1-3 (with ci=0 attn tiles woven
        # in), then attention rows 1-3 merged with the out-projection -----
        if True:
            psum_pools = {}
            tri = mask_sb[:, 0, :]
            ident = mask_sb[:, 1, :]

            # ---- attention tile helpers (used for ci=0 during proj and
            # for rows 1..3 in the attention phase) ----
            def emit_qk_pair(ci, h, pt, pi):
                npairs = 2 * ci + 2
                bj0, bj1 = 2 * pi, 2 * pi + 1
                ps = psum_pools["qk"].tile([P, 2, 512], F32, tag="qkps")
                for u, bj in ((0, bj0), (1, bj1)):
                    rr = bj - 4 * ci
                    if rr <= 0:
                        nc.tensor.matmul(
                            ps[:, u, :],
                            lhsT=ks[:, h, bj * P:(bj + 1) * P],
                            rhs=qs[:, h, ci * 512:(ci + 1) * 512],
                            start=True, stop=True,
                        )
                    else:
                        nc.tensor.matmul(
                            ps[:, u, rr * P:],
                            lhsT=ks[:, h, bj * P:(bj + 1) * P],
                            rhs=qs[:, h, ci * 512 + rr * P:(ci + 1) * 512],
                            start=True, stop=True,
                        )
                if pi == npairs - 1:
                    # diagonal pair: only causally-valid columns
                    nc.scalar.activation(
                        out=pt[:, bj0, 256:], in_=ps[:, 0, 256:], func=EXPF)
                    nc.scalar.activation(
                        out=pt[:, bj1, 384:], in_=ps[:, 1, 384:], func=EXPF)
                else:
                    nc.scalar.activation(
                        out=pt[:, bj0:bj0 + 2, :], in_=ps, func=EXPF)
                # triangular mask on strictly-diagonal 128x128 squares
                for u, bj in ((0, bj0), (1, bj1)):
                    rr = bj - 4 * ci
                    if rr >= 0:
                        sq = slice(rr * P, (rr + 1) * P)
                        nc.vector.tensor_mul(
                            out=pt[:, bj, sq], in0=pt[:, bj, sq], in1=tri)

            def av_item(ci, h, ib, pt, att_h):
                gi = 4 * ci + ib
                avp = psum_pools["av"].tile([P, D + 4], F32, tag="avps")
                isl = slice(ib * P, (ib + 1) * P)
                for bj in range(gi + 1):
                    nc.tensor.matmul(
                        avp[:, :D + 1],
                        lhsT=pt[:, bj, isl],
                        rhs=v_all[:, bj, h, :D + 1],
                        start=(bj == 0), stop=(bj == gi),
                    )
                rs = rs_pool.tile([P, 1], F32, tag="rs")
                nc.vector.reciprocal_approx_fast(out=rs, in_=avp[:, D:D + 1])
                nc.vector.tensor_scalar_mul(
                    out=att_h[:, ib, :], in0=avp[:, :D], scalar1=rs)
                if h == HPC - 1 and ci > 0:
                    # the last head's transpose gates the whole out-proj
                    # group: do it on the PE (identity matmul) so the group
                    # unlocks right after the eviction
                    tp = psum_pools["av"].tile([P, D], FP16, tag="avps",
                                               name="tps")
                    nc.tensor.transpose(tp, att_h[:, ib, :], ident)
                    nc.vector.tensor_copy(
                        out=outTs[h][:, ci * 512 + ib * P:
                                     ci * 512 + (ib + 1) * P],
                        in_=tp)
                elif ib == 3:
                    # non-gating heads: DRAM round-trip transpose (zero PE
                    # cost); lands several tiles before the group unlocks
                    nsl = slice(ci * 512, (ci + 1) * 512)
                    nc.sync.dma_start(
                        out=attd[h, nsl, :].rearrange(
                            "(io p) d -> p io d", p=P),
                        in_=att_h)
                    nc.sync.dma_start_transpose(
                        out=outTs[h][:, nsl], in_=attd[h, nsl, :])

            # ---- proj nch 1..3 with ci=0 tiles woven between chains ----
            with (
                tc.tile_pool(name="pj2", bufs=3, space="PSUM") as pj2,
                tc.tile_pool(name="qkw", bufs=1, space="PSUM") as qkw,
                tc.tile_pool(name="avw", bufs=1, space="PSUM") as avw,
            ):
                psum_pools["qk"] = qkw
                psum_pools["av"] = avw
                pts0 = [pt_pool.tile([P, ET, 512], FP16, tag="pt",
                                     name=f"pt0{h}") for h in range(HPC)]
                atts0 = [att_pool.tile([P, NCH, D], FP16, tag="atth",
                                       name=f"att0{h}") for h in range(HPC)]
                # (kind, args): proj chain steps interleaved with ci=0 work
                weave = deque()
                for h in range(HPC):
                    weave.append(("qk", h, 0))
                    weave.append(("qk", h, 1))
                    for ib in range(4):
                        weave.append(("av", h, ib))

                def weave_step(budget):
                    # pop ci=0 attn pieces; each is tiny vs a proj chain
                    n = 0
                    while weave and n < budget:
                        kind, h, idx = weave[0]
                        if kind == "qk":
                            emit_qk_pair(0, h, pts0[h], idx)
                        else:
                            av_item(0, h, idx, pts0[h], atts0[h])
                        weave.popleft()
                        n += 1

                for nch in range(1, NCH):
                    x_sb = x_tiles[nch]
                    nsl = slice(nch * 512, (nch + 1) * 512)

                    for w_sb, dst in ((wq_sb, qs), (wk_sb, ks)):
                        for t in range(HPC):
                            if nch == 1 and w_sb is wq_sb and t == 0:
                                continue
                            ps = pj2.tile([P, 512], F32, tag="pjps")
                            for et in range(ET):
                                nc.tensor.matmul(
                                    ps,
                                    lhsT=w_sb[:, et, t * P:(t + 1) * P],
                                    rhs=x_sb[:, et, :],
                                    start=(et == 0), stop=(et == ET - 1),
                                )
                            if nch == NCH - 1:
                                nc.scalar.copy(out=dst[:, t, nsl], in_=ps)
                            else:
                                nc.vector.tensor_copy(
                                    out=dst[:, t, nsl], in_=ps)
                            weave_step(1)

                    for nb in range(4):
                        ps = pj2.tile([P, 512], F32, tag="pjps")
                        for et in range(ET):
                            nc.tensor.matmul(
                                ps,
                                lhsT=x_sb[:, et, nb * P:(nb + 1) * P],
                                rhs=wv_sb[:, et, :],
                                start=(et == 0), stop=(et == ET - 1),
                            )
                        if nch == NCH - 1:
                            nc.scalar.copy(
                                out=v_all[:, nch * 4 + nb, :, :D],
                                in_=ps.rearrange("p (h d) -> p h d", h=HPC))
                        else:
                            nc.vector.tensor_copy(
                                out=v_all[:, nch * 4 + nb, :, :D],
                                in_=ps.rearrange("p (h d) -> p h d", h=HPC))
                        weave_step(1)

                    if nch + 2 < NCH:
                        load_x(nch + 2)
                weave_step(99)
            xpool.release()
            wpool.release()

            # ---- attention rows 1..3 + out-projection fill ----
            with (
                tc.tile_pool(name="qk_ps", bufs=2, space="PSUM") as qk_ps,
                tc.tile_pool(name="av_ps", bufs=2, space="PSUM") as av_ps,
                tc.tile_pool(name="op_ps", bufs=2, space="PSUM") as op_ps,
                tc.tile_pool(name="op_ev", bufs=3) as op_ev,
            ):
                psum_pools["qk"] = qk_ps
                psum_pools["av"] = av_ps
                favq = deque()     # (cost_ns, emit_fn) A@V of the prev tile
                fopq = deque()     # (cost_ns, emit_fn) out-proj items
                transposed = [HPC, 0, 0, 0]
                tdone_step = [-99, None, None, None]
                op_queued = [False] * NCH
                op_state = {}      # nb -> ostage tile
                evict_flip = [0]
                step = [0]

                def make_op_item(nb, ec):
                    def emit():
                        if ec == 0:
                            op_state[nb] = op_ev.tile(
                                [P, NCH, 512], FP16, tag="opev",
                                name=f"ost{nb}")
                        ostage = op_state[nb]
                        ps = op_ps.tile([P, 512], F32, tag="opps")
                        for t in range(HPC):
                            nc.tensor.matmul(
                                ps,
                                lhsT=outTs[t][:, nb * P:(nb + 1) * P],
                                rhs=wo_sb[:, t, ec * 512:(ec + 1) * 512],
                                start=(t == 0), stop=(t == HPC - 1),
                            )
                        if evict_flip[0] == 0:
                            nc.vector.tensor_copy(out=ostage[:, ec, :],
                                                  in_=ps)
                        else:
                            nc.scalar.copy(out=ostage[:, ec, :], in_=ps)
                        evict_flip[0] ^= 1
                        if nb == 4 * NCH - 1:
                            # very last row-block: per-ec DMAs so the final
                            # transfer trailing the last matmul is small
                            nc.sync.dma_start(
                                out=out.ap()[nb * P:(nb + 1) * P,
                                             ec * 512:(ec + 1) * 512],
                                in_=ostage[:, ec, :])
                        elif nb >= 4 * (NCH - 1):
                            if ec == 1:
                                nc.sync.dma_start(
                                    out=out.ap()[nb * P:(nb + 1) * P,
                                                 0:1024],
                                    in_=ostage[:, 0:2, :])
                            elif ec == 3:
                                nc.sync.dma_start(
                                    out=out.ap()[nb * P:(nb + 1) * P,
                                                 1024:2048],
                                    in_=ostage[:, 2:4, :])
                        elif ec == NCH - 1:
                            nc.sync.dma_start(
                                out=out.ap()[nb * P:(nb + 1) * P, :],
                                in_=ostage)
                    return emit

                def queue_ready_op():
                    for cig in range(NCH):
                        if op_queued[cig] or transposed[cig] < HPC:
                            continue
                        if cig > 0 and step[0] < tdone_step[cig] + 1:
                            continue
                        op_queued[cig] = True
                        for nb in range(cig * 4, cig * 4 + 4):
                            for ec in range(NCH):
                                fopq.append((4 * 213, make_op_item(nb, ec)))

                def emit_fill(target_ns, prefer_op):
                    acc = 0
                    while acc < target_ns:
                        if prefer_op and fopq:
                            q = fopq
                        elif favq:
                            q = favq
                        elif fopq:
                            q = fopq
                        else:
                            return
                        cost, fn = q.popleft()
                        fn()
                        acc += cost
                        prefer_op = False

                def mark_transposed(ci):
                    transposed[ci] += 1
                    if transposed[ci] == HPC:
                        tdone_step[ci] = step[0]

                prev = None  # (ci, h, pt, att_h)
                for ci in range(1, NCH):
                    for h in range(HPC):
                        queue_ready_op()
                        if prev is not None:
                            pci, ph, ppt, patt = prev
                            for ib in range(4):
                                def mk(pci=pci, ph=ph, ib=ib, ppt=ppt,
                                       patt=patt):
                                    def em():
                                        av_item(pci, ph, ib, ppt, patt)
                                        if ib == 3:
                                            mark_transposed(pci)
                                    return em
                                favq.append(
                                    ((4 * pci + ib + 1) * 54 + 150, mk()))
                        pt = pt_pool.tile([P, ET, 512], FP16, tag="pt")
                        att_h = att_pool.tile([P, NCH, D], FP16, tag="atth")
                        npairs = 2 * ci + 2
                        for pi in range(npairs):
                            # fill BEFORE the pair: the pair's PSUM bank is
                            # gated by an earlier pair's exp, and the PE is
                            # in-order — fill emitted after a stalled matmul
                            # would be stuck behind it
                            emit_fill(
                                400 if pi == npairs - 1 else 800,
                                prefer_op=(pi == 0))
                            emit_qk_pair(ci, h, pt, pi)
                        prev = (ci, h, pt, att_h)
                        step[0] += 1

                # drain: last tile's A@V with PE-side transposes, then the
                # remaining out-proj chunks
                pci, ph, ppt, patt = prev
                while favq:
                    favq.popleft()[1]()
                for ib in range(4):
                    av_item(pci, ph, ib, ppt, patt)
                transposed[pci] = HPC
                tdone_step[pci] = step[0] - 2
                queue_ready_op()
                while fopq:
                    fopq.popleft()[1]()
                step[0] += 4
                queue_ready_op()
                while fopq:
                    fopq.popleft()[1]()

        rs_pool.release()
        att_pool.release()
        pt_pool.release()
        for _pl in reversed(_longlived):
            _pl.release()

    nc.compile()
    return nc


def make_in_maps(x, Wq, Wkv, Wout):
    x = np.asarray(x, dtype=np.float32)
    Wq = np.asarray(Wq, dtype=np.float32)
    Wkv = np.asarray(Wkv, dtype=np.float32)
    Wout = np.asarray(Wout, dtype=np.float32)
    scale = np.float32(D ** -0.5)

    # [strictly-diagonal causal mask (col >= row) | identity]
    jj = np.arange(P)[:, None]
    ii = np.arange(P)[None, :]
    mask = np.zeros((P, 2, P), dtype=np.float16)
    mask[:, 0, :] = (ii >= jj).astype(np.float16)
    mask[:, 1, :] = (ii == jj).astype(np.float16)

    xT = [np.ascontiguousarray(x[b].T).astype(np.float16) for b in range(B)]
    in_maps = []
    for c in range(NCORES):
        b, hg = divmod(c, 4)
        sl = slice(hg * DC, (hg + 1) * DC)
        in_maps.append({
            "xT": xT[b],
            "wqT": (np.ascontiguousarray(Wq[sl, :].T) * scale).astype(np.float16),
            "wkT": np.ascontiguousarray(Wkv[sl, :].T).astype(np.float16),
            "wvT": np.ascontiguousarray(Wkv[E + sl.start:E + sl.stop, :].T).astype(np.float16),
            "woT": np.ascontiguousarray(Wout[:, sl].T).astype(np.float16),
            "maskin": mask,
        })
    return in_maps


_NC_CACHE = []


def _get_nc():
    if not _NC_CACHE:
        _NC_CACHE.append(build_nc())
    return _NC_CACHE[0]


def _run(in_maps):
    nc = _get_nc()
    return run_bass_kernel_spmd(nc, in_maps, core_ids=list(range(NCORES)))


def kernel(x, Wq, Wkv, Wout):
    in_maps = make_in_maps(x, Wq, Wkv, Wout)
    res = _run(in_maps)
    out = np.zeros((B, N, E), dtype=np.float32)
    for c in range(NCORES):
        out[c // 4] += res.results[c]["out"].astype(np.float32)
    return out


if __name__ == "__main__":
    t0 = time.time()
    _get_nc()
    print(f"build+compile: {time.time() - t0:.1f}s")


# revision 41
# speedup vs baseline: 1.0974x; 1.0047x over previous
"""Trainium2 Bass kernel for nn_BaseAttention (B=2, N=2048, E=2048, H=16, D=128).

Sharding: 8 cores; core c handles batch b=c//4, head-group hg=c%4 (4 heads).
Each core computes q/k/v projections for its heads, causal flash-style
attention, and a partial out-projection (contraction over its 512 head dims).
Host sums the 4 partial outputs per batch (tensor-parallel unshard).

Schedule (v3):
- QK^T and A@V are causally exact at 128-block granularity; only the
  strictly-diagonal 128x128 squares get a triangular mask multiply.
- The four ci=0 attention tiles (which need only chunk 0 of q/k/v) are woven
  into the projection phase, so their exp/spill/transpose chain finishes long
  before the attention phase starts.
- In the attention phase, out-projection matmuls and the previous tile's A@V
  are emitted as PE fill work BEFORE each QK PSUM pair, so the in-order PE
  never parks on the ScalarE exp cadence.
- The last attention row's outputs are transposed on the PE (via identity
  matmul) instead of the DRAM round-trip, removing the final transpose DMA
  latency from the critical path.
- A dummy warmup matmul chain absorbs the PE p-state ramp while the first
  input DMA pieces land; the first projection chunk runs et-outer so the PE
  starts consuming pieces as they arrive.
- Output is written fp16 and summed on host in fp32.
"""

import sys
import time

sys.path.insert(0, "/opt/trn_rl_repo")

from collections import deque

import numpy as np

import concourse.bass as bass
import concourse.mybir as mybir
import concourse.tile as tile
from concourse import bacc
from concourse.bass_utils import run_bass_kernel_spmd

B, N, E, H = 2, 2048, 2048, 16
D = E // H            # 128
HPC = 4               # heads per core
DC = HPC * D          # 512 head dims per core
NCORES = 8
P = 128
NCH = N // 512        # 4 n-chunks of 512
ET = E // P           # 16 e-tiles of 128

F32 = mybir.dt.float32
FP16 = mybir.dt.float16


def build_nc():
    nc = bacc.Bacc("TRN2", target_bir_lowering=False, debug=False,
                   num_devices=NCORES)

    xT = nc.dram_tensor("xT", [E, N], FP16, kind="ExternalInput")
    wqT = nc.dram_tensor("wqT", [E, DC], FP16, kind="ExternalInput")
    wkT = nc.dram_tensor("wkT", [E, DC], FP16, kind="ExternalInput")
    wvT = nc.dram_tensor("wvT", [E, DC], FP16, kind="ExternalInput")
    woT = nc.dram_tensor("woT", [DC, E], FP16, kind="ExternalInput")
    maskin = nc.dram_tensor("maskin", [P, 2, P], FP16, kind="ExternalInput")
    out = nc.dram_tensor("out", [N, E], FP16, kind="ExternalOutput")

    xT_r = xT.ap().rearrange("(eo p) n -> p eo n", p=P)      # [128,16,2048]
    wqT_r = wqT.ap().rearrange("(eo p) d -> p eo d", p=P)    # [128,16,512]
    wkT_r = wkT.ap().rearrange("(eo p) d -> p eo d", p=P)
    wvT_r = wvT.ap().rearrange("(eo p) d -> p eo d", p=P)
    woT_r = woT.ap().rearrange("(t p) e -> p t e", p=P)      # [128,4,2048]

    EXPF = mybir.ActivationFunctionType.Exp

    with tile.TileContext(nc) as tc:
        # ---------------- constants + spill tensors ----------------
        consts = tc.alloc_tile_pool(name="consts", bufs=1)
        _longlived = [consts]
        mask_sb = consts.tile([P, 2, P], FP16)   # [tri(c>=p) | identity]
        warm_sb = consts.tile([P, 512], FP16)
        # prefire the Exp table load so it overlaps the input DMA head
        dummy = consts.tile([1, 8], F32)
        nc.gpsimd.memset(warm_sb, 0.0)
        nc.vector.memset(dummy, 0.0)
        nc.scalar.activation(out=dummy, in_=dummy, func=EXPF)

        dram = tc.alloc_tile_pool(name="dram", bufs=1, space="DRAM")
        _longlived.append(dram)
        attd = dram.tile([HPC, N, D], FP16)          # normalized attn out

        # per-core activations, SBUF-resident across the whole kernel
        big = tc.alloc_tile_pool(name="big", bufs=1)
        _longlived.append(big)
        qs = big.tile([P, HPC, N], FP16)                  # q^T, heads stacked
        ks = big.tile([P, HPC, N], FP16)                  # k^T
        v_all = big.tile([P, N // P, HPC, D + 4], FP16)   # [V | 1] per block
        nc.vector.memset(v_all[:, :, :, D:D + 1], 1.0)

        outT_pool = tc.alloc_tile_pool(name="outT", bufs=1)
        _longlived.append(outT_pool)
        outTs = [outT_pool.tile([P, N], FP16, name=f"outT{t}")
                 for t in range(HPC)]
        wo_pool = tc.alloc_tile_pool(name="wo_pool", bufs=1)
        _longlived.append(wo_pool)
        wo_sb = wo_pool.tile([P, HPC, E], FP16)

        pt_pool = tc.alloc_tile_pool(name="pt_pool", bufs=2)
        att_pool = tc.alloc_tile_pool(name="att_pool", bufs=3)
        rs_pool = tc.alloc_tile_pool(name="rs_pool", bufs=8)

        # ---------------- phase 1a: nch-0 projections (et-outer) --------
        wpool = tc.alloc_tile_pool(name="wpool", bufs=1)
        xpool = tc.alloc_tile_pool(name="xpool", bufs=2)
        wq_sb = wpool.tile([P, ET, DC], FP16)
        wk_sb = wpool.tile([P, ET, DC], FP16)
        wv_sb = wpool.tile([P, ET, DC], FP16)
        x_tiles = [None] * NCH

        def load_x(nch):
            t = xpool.tile([P, ET, 512], FP16, tag="xchunk",
                           name=f"x_sb{nch}")
            nc.sync.dma_start(
                out=t, in_=xT_r[:, :, nch * 512:(nch + 1) * 512])
            x_tiles[nch] = t

        warmp = tc.alloc_tile_pool(name="warmp", bufs=1, space="PSUM")
        pj8 = tc.alloc_tile_pool(name="pj8", bufs=4, space="PSUM")
        qkw = tc.alloc_tile_pool(name="qkw", bufs=1, space="PSUM")
        avw = tc.alloc_tile_pool(name="avw", bufs=1, space="PSUM")
        if True:
            # PE warmup: absorb the p-state ramp on dummy matmuls while the
            # first input pieces stream in
            warm_ps = warmp.tile([P, 512], F32, tag="warm")
            for w in range(7):
                nc.tensor.matmul(warm_ps, lhsT=warm_sb[:, 0:P],
                                 rhs=warm_sb, start=(w == 0), stop=(w == 6))

            # small pieces throughout: PE consumption (~0.85us/et) only just
            # trails DMA supply (~0.72us/et), so a late big piece stalls PE
            x0 = xpool.tile([P, ET, 512], FP16, tag="xchunk", name="x_sb0")
            x_tiles[0] = x0
            for a, b in ((0, 1), (1, 2), (2, 4), (4, 6), (6, 8), (8, 10),
                         (10, 12), (12, 14), (14, 16)):
                gs = slice(a, b)
                nc.sync.dma_start(out=wq_sb[:, gs, :], in_=wqT_r[:, gs, :])
                nc.sync.dma_start(out=x0[:, gs, :], in_=xT_r[:, gs, 0:512])
            for g in range(4):
                gs = slice(g * 4, (g + 1) * 4)
                nc.sync.dma_start(out=wk_sb[:, gs, :], in_=wkT_r[:, gs, :])
            load_x(1)
            nc.sync.dma_start(out=mask_sb, in_=maskin.ap())
            HF = ET // 2
            nc.sync.dma_start(out=wv_sb[:, :HF, :], in_=wvT_r[:, :HF, :])
            nc.sync.dma_start(out=wv_sb[:, HF:, :], in_=wvT_r[:, HF:, :])

            for w_sb, dst in ((wq_sb, qs), (wk_sb, ks)):
                pss = [pj8.tile([P, 512], F32, tag="pjps", name=f"pjt{t}")
                       for t in range(HPC)]
                for et in range(ET):
                    for t in range(HPC):
                        nc.tensor.matmul(
                            pss[t],
                            lhsT=w_sb[:, et, t * P:(t + 1) * P],
                            rhs=x0[:, et, :],
                            start=(et == 0), stop=(et == ET - 1),
                        )
                for t in range(HPC):
                    nc.vector.tensor_copy(out=dst[:, t, 0:512], in_=pss[t])
            for nb in range(4):
                ps = pj8.tile([P, 512], F32, tag="pjps")
                for et in range(ET):
                    nc.tensor.matmul(
                        ps,
                        lhsT=x0[:, et, nb * P:(nb + 1) * P],
                        rhs=wv_sb[:, et, :],
                        start=(et == 0), stop=(et == ET - 1),
                    )
                nc.vector.tensor_copy(
                    out=v_all[:, nb, :, :D],
                    in_=ps.rearrange("p (h d) -> p h d", h=HPC))
                if nb == 2:
                    # first nch-1 q chain here: its PE work hides the last
                    # v eviction that gates the next pool's coarse sems
                    ps_q1 = pj8.tile([P, 512], F32, tag="pjps",
                                     name="psq1")
                    for et in range(ET):
                        nc.tensor.matmul(
                            ps_q1,
                            lhsT=wq_sb[:, et, 0:P],
                            rhs=x_tiles[1][:, et, :],
                            start=(et == 0), stop=(et == ET - 1),
                        )
                    nc.vector.tensor_copy(out=qs[:, 0, 512:1024],
                                          in_=ps_q1)
            load_x(2)
            # wo loads here (DMA slack mid-proj) so the attention-phase DMA
            # queue is free for the attention spills/transposes
            for t in range(HPC):
                nc.sync.dma_start(out=wo_sb[:, t, :], in_=woT_r[:, t, :])

        # ------- phase 1b + 2: proj nch 1-3 (with ci=0 attn tiles woven
        # in), then attention rows 1-3 merged with the out-projection -----
        if True:
            psum_pools = {}
            tri = mask_sb[:, 0, :]
            ident = mask_sb[:, 1, :]

            # ---- attention tile helpers (used for ci=0 during proj and
            # for rows 1..3 in the attention phase) ----
            def emit_qk_pair(ci, h, pt, pi):
                npairs = 2 * ci + 2
                bj0, bj1 = 2 * pi, 2 * pi + 1
                ps = psum_pools["qk"].tile([P, 2, 512], F32, tag="qkps")
                for u, bj in ((0, bj0), (1, bj1)):
                    rr = bj - 4 * ci
                    if rr <= 0:
                        nc.tensor.matmul(
                            ps[:, u, :],
                            lhsT=ks[:, h, bj * P:(bj + 1) * P],
                            rhs=qs[:, h, ci * 512:(ci + 1) * 512],
                            start=True, stop=True,
                        )
                    else:
                        nc.tensor.matmul(
                            ps[:, u, rr * P:],
                            lhsT=ks[:, h, bj * P:(bj + 1) * P],
                            rhs=qs[:, h, ci * 512 + rr * P:(ci + 1) * 512],
                            start=True, stop=True,
                        )
                if pi == npairs - 1:
                    # diagonal pair: only causally-valid columns
                    nc.scalar.activation(
                        out=pt[:, bj0, 256:], in_=ps[:, 0, 256:], func=EXPF)
                    nc.scalar.activation(
                        out=pt[:, bj1, 384:], in_=ps[:, 1, 384:], func=EXPF)
                else:
                    nc.scalar.activation(
                        out=pt[:, bj0:bj0 + 2, :], in_=ps, func=EXPF)
                # triangular mask on strictly-diagonal 128x128 squares
                for u, bj in ((0, bj0), (1, bj1)):
                    rr = bj - 4 * ci
                    if rr >= 0:
                        sq = slice(rr * P, (rr + 1) * P)
                        nc.vector.tensor_mul(
                            out=pt[:, bj, sq], in0=pt[:, bj, sq], in1=tri)

            def av_item(ci, h, ib, pt, att_h):
                gi = 4 * ci + ib
                avp = psum_pools["av"].tile([P, D + 4], F32, tag="avps")
                isl = slice(ib * P, (ib + 1) * P)
                for bj in range(gi + 1):
                    nc.tensor.matmul(
                        avp[:, :D + 1],
                        lhsT=pt[:, bj, isl],
                        rhs=v_all[:, bj, h, :D + 1],
                        start=(bj == 0), stop=(bj == gi),
                    )
                rs = rs_pool.tile([P, 1], F32, tag="rs")
                nc.vector.reciprocal_approx_fast(out=rs, in_=avp[:, D:D + 1])
                nc.vector.tensor_scalar_mul(
                    out=att_h[:, ib, :], in0=avp[:, :D], scalar1=rs)
                if h == HPC - 1 and ci > 0:
                    # the last head's transpose gates the whole out-proj
                    # group: do it on the PE (identity matmul) so the group
                    # unlocks right after the eviction
                    tp = psum_pools["av"].tile([P, D], FP16, tag="avps",
                                               name="tps")
                    nc.tensor.transpose(tp, att_h[:, ib, :], ident)
                    nc.vector.tensor_copy(
                        out=outTs[h][:, ci * 512 + ib * P:
                                     ci * 512 + (ib + 1) * P],
                        in_=tp)
                elif ib == 3:
                    # non-gating heads: DRAM round-trip transpose (zero PE
                    # cost); lands several tiles before the group unlocks
                    nsl = slice(ci * 512, (ci + 1) * 512)
                    nc.sync.dma_start(
                        out=attd[h, nsl, :].rearrange(
                            "(io p) d -> p io d", p=P),
                        in_=att_h)
                    nc.sync.dma_start_transpose(
                        out=outTs[h][:, nsl], in_=attd[h, nsl, :])

            # ---- proj nch 1..3 with ci=0 tiles woven between chains ----
            if True:
                psum_pools["qk"] = qkw
                psum_pools["av"] = avw
                pts0 = [pt_pool.tile([P, ET, 512], FP16, tag="pt",
                                     name=f"pt0{h}") for h in range(HPC)]
                atts0 = [att_pool.tile([P, NCH, D], FP16, tag="atth",
                                       name=f"att0{h}") for h in range(HPC)]
                # (kind, args): proj chain steps interleaved with ci=0 work
                weave = deque()
                for h in range(HPC):
                    weave.append(("qk", h, 0))
                    weave.append(("qk", h, 1))
                    for ib in range(4):
                        weave.append(("av", h, ib))

                def weave_step(budget):
                    # pop ci=0 attn pieces; each is tiny vs a proj chain
                    n = 0
                    while weave and n < budget:
                        kind, h, idx = weave[0]
                        if kind == "qk":
                            emit_qk_pair(0, h, pts0[h], idx)
                        else:
                            av_item(0, h, idx, pts0[h], atts0[h])
                        weave.popleft()
                        n += 1

                for nch in range(1, NCH):
                    x_sb = x_tiles[nch]
                    nsl = slice(nch * 512, (nch + 1) * 512)

                    for w_sb, dst in ((wq_sb, qs), (wk_sb, ks)):
                        for t in range(HPC):
                            if nch == 1 and w_sb is wq_sb and t == 0:
                                continue
                            ps = pj8.tile([P, 512], F32, tag="pjps")
                            for et in range(ET):
                                nc.tensor.matmul(
                                    ps,
                                    lhsT=w_sb[:, et, t * P:(t + 1) * P],
                                    rhs=x_sb[:, et, :],
                                    start=(et == 0), stop=(et == ET - 1),
                                )
                            nc.vector.tensor_copy(
                                out=dst[:, t, nsl], in_=ps)
                            weave_step(1)

                    for nb in range(4):
                        ps = pj8.tile([P, 512], F32, tag="pjps")
                        for et in range(ET):
                            nc.tensor.matmul(
                                ps,
                                lhsT=x_sb[:, et, nb * P:(nb + 1) * P],
                                rhs=wv_sb[:, et, :],
                                start=(et == 0), stop=(et == ET - 1),
                            )
                        nc.vector.tensor_copy(
                            out=v_all[:, nch * 4 + nb, :, :D],
                            in_=ps.rearrange("p (h d) -> p h d", h=HPC))
                        weave_step(1)

                    if nch + 2 < NCH:
                        load_x(nch + 2)
                weave_step(99)
            avw.release()
            qkw.release()
            pj8.release()
            warmp.release()
            xpool.release()
            wpool.release()

            # ---- attention rows 1..3 + out-projection fill ----
            with (
                tc.tile_pool(name="qk_ps", bufs=2, space="PSUM") as qk_ps,
                tc.tile_pool(name="av_ps", bufs=2, space="PSUM") as av_ps,
                tc.tile_pool(name="op_ps", bufs=2, space="PSUM") as op_ps,
                tc.tile_pool(name="op_ev", bufs=3) as op_ev,
            ):
                psum_pools["qk"] = qk_ps
                psum_pools["av"] = av_ps
                favq = deque()     # (cost_ns, emit_fn) A@V of the prev tile
                fopq = deque()     # (cost_ns, emit_fn) out-proj items
                transposed = [HPC, 0, 0, 0]
                tdone_step = [-99, None, None, None]
                op_queued = [False] * NCH
                op_state = {}      # nb -> ostage tile
                evict_flip = [0]
                step = [0]

                def make_op_item(nb, ec):
                    def emit():
                        if ec == 0:
                            op_state[nb] = op_ev.tile(
                                [P, NCH, 512], FP16, tag="opev",
                                name=f"ost{nb}")
                        ostage = op_state[nb]
                        ps = op_ps.tile([P, 512], F32, tag="opps")
                        for t in range(HPC):
                            nc.tensor.matmul(
                                ps,
                                lhsT=outTs[t][:, nb * P:(nb + 1) * P],
                                rhs=wo_sb[:, t, ec * 512:(ec + 1) * 512],
                                start=(t == 0), stop=(t == HPC - 1),
                            )
                        nc.vector.tensor_copy(out=ostage[:, ec, :],
                                              in_=ps)
                        if nb == 4 * NCH - 1:
                            # very last row-block: per-ec DMAs so the final
                            # transfer trailing the last matmul is small
                            nc.sync.dma_start(
                                out=out.ap()[nb * P:(nb + 1) * P,
                                             ec * 512:(ec + 1) * 512],
                                in_=ostage[:, ec, :])
                        elif nb >= 4 * (NCH - 1):
                            if ec == 1:
                                nc.sync.dma_start(
                                    out=out.ap()[nb * P:(nb + 1) * P,
                                                 0:1024],
                                    in_=ostage[:, 0:2, :])
                            elif ec == 3:
                                nc.sync.dma_start(
                                    out=out.ap()[nb * P:(nb + 1) * P,
                                                 1024:2048],
                                    in_=ostage[:, 2:4, :])
                        elif ec == NCH - 1:
                            nc.sync.dma_start(
                                out=out.ap()[nb * P:(nb + 1) * P, :],
                                in_=ostage)
                    return emit

                def queue_ready_op():
                    for cig in range(NCH):
                        if op_queued[cig] or transposed[cig] < HPC:
                            continue
                        if cig > 0 and step[0] < tdone_step[cig] + 1:
                            continue
                        op_queued[cig] = True
                        for nb in range(cig * 4, cig * 4 + 4):
                            for ec in range(NCH):
                                fopq.append((4 * 213, make_op_item(nb, ec)))

                def emit_fill(target_ns, prefer_op):
                    acc = 0
                    while acc < target_ns:
                        if prefer_op and fopq:
                            q = fopq
                        elif favq:
                            q = favq
                        elif fopq:
                            q = fopq
                        else:
                            return
                        cost, fn = q.popleft()
                        fn()
                        acc += cost
                        prefer_op = False

                def mark_transposed(ci):
                    transposed[ci] += 1
                    if transposed[ci] == HPC:
                        tdone_step[ci] = step[0]

                prev = None  # (ci, h, pt, att_h)
                for ci in range(1, NCH):
                    for h in range(HPC):
                        queue_ready_op()
                        if prev is not None:
                            pci, ph, ppt, patt = prev
                            for ib in range(4):
                                def mk(pci=pci, ph=ph, ib=ib, ppt=ppt,
                                       patt=patt):
                                    def em():
                                        av_item(pci, ph, ib, ppt, patt)
                                        if ib == 3:
                                            mark_transposed(pci)
                                    return em
                                favq.append(
                                    ((4 * pci + ib + 1) * 54 + 150, mk()))
                        pt = pt_pool.tile([P, ET, 512], FP16, tag="pt")
                        att_h = att_pool.tile([P, NCH, D], FP16, tag="atth")
                        npairs = 2 * ci + 2
                        for pi in range(npairs):
                            # fill BEFORE the pair: the pair's PSUM bank is
                            # gated by an earlier pair's exp, and the PE is
                            # in-order — fill emitted after a stalled matmul
                            # would be stuck behind it
                            emit_fill(
                                400 if pi == npairs - 1 else 800,
                                prefer_op=(pi == 0))
                            emit_qk_pair(ci, h, pt, pi)
                        prev = (ci, h, pt, att_h)
                        step[0] += 1

                # drain: last tile's A@V with PE-side transposes, then the
                # remaining out-proj chunks
                pci, ph, ppt, patt = prev
                while favq:
                    favq.popleft()[1]()
                for ib in range(4):
                    av_item(pci, ph, ib, ppt, patt)
                transposed[pci] = HPC
                tdone_step[pci] = step[0] - 2
                queue_ready_op()
                while fopq:
                    fopq.popleft()[1]()
                step[0] += 4
                queue_ready_op()
                while fopq:
                    fopq.popleft()[1]()

        rs_pool.release()
        att_pool.release()
        pt_pool.release()
        for _pl in reversed(_longlived):
            _pl.release()

    nc.compile()
    return nc


def make_in_maps(x, Wq, Wkv, Wout):
    x = np.asarray(x, dtype=np.float32)
    Wq = np.asarray(Wq, dtype=np.float32)
    Wkv = np.asarray(Wkv, dtype=np.float32)
    Wout = np.asarray(Wout, dtype=np.float32)
    scale = np.float32(D ** -0.5)

    # [strictly-diagonal causal mask (col >= row) | identity]
    jj = np.arange(P)[:, None]
    ii = np.arange(P)[None, :]
    mask = np.zeros((P, 2, P), dtype=np.float16)
    mask[:, 0, :] = (ii >= jj).astype(np.float16)
    mask[:, 1, :] = (ii == jj).astype(np.float16)

    xT = [np.ascontiguousarray(x[b].T).astype(np.float16) for b in range(B)]
    in_maps = []
    for c in range(NCORES):
        b, hg = divmod(c, 4)
        sl = slice(hg * DC, (hg + 1) * DC)
        in_maps.append({
            "xT": xT[b],
            "wqT": (np.ascontiguousarray(Wq[sl, :].T) * scale).astype(np.float16),
            "wkT": np.ascontiguousarray(Wkv[sl, :].T).astype(np.float16),
            "wvT": np.ascontiguousarray(Wkv[E + sl.start:E + sl.stop, :].T).astype(np.float16),
            "woT": np.ascontiguousarray(Wout[:, sl].T).astype(np.float16),
            "maskin": mask,
        })
    return in_maps


_NC_CACHE = []


def _get_nc():
    if not _NC_CACHE:
        _NC_CACHE.append(build_nc())
    return _NC_CACHE[0]


def _run(in_maps):
    nc = _get_nc()
    return run_bass_kernel_spmd(nc, in_maps, core_ids=list(range(NCORES)))


def kernel(x, Wq, Wkv, Wout):
    in_maps = make_in_maps(x, Wq, Wkv, Wout)
    res = _run(in_maps)
    out = np.zeros((B, N, E), dtype=np.float32)
    for c in range(NCORES):
        out[c // 4] += res.results[c]["out"].astype(np.float32)
    return out


if __name__ == "__main__":
    t0 = time.time()
    _get_nc()
    print(f"build+compile: {time.time() - t0:.1f}s")


# revision 42
# speedup vs baseline: 1.0982x; 1.0007x over previous
"""Trainium2 Bass kernel for nn_BaseAttention (B=2, N=2048, E=2048, H=16, D=128).

Sharding: 8 cores; core c handles batch b=c//4, head-group hg=c%4 (4 heads).
Each core computes q/k/v projections for its heads, causal flash-style
attention, and a partial out-projection (contraction over its 512 head dims).
Host sums the 4 partial outputs per batch (tensor-parallel unshard).

Schedule (v3):
- QK^T and A@V are causally exact at 128-block granularity; only the
  strictly-diagonal 128x128 squares get a triangular mask multiply.
- The four ci=0 attention tiles (which need only chunk 0 of q/k/v) are woven
  into the projection phase, so their exp/spill/transpose chain finishes long
  before the attention phase starts.
- In the attention phase, out-projection matmuls and the previous tile's A@V
  are emitted as PE fill work BEFORE each QK PSUM pair, so the in-order PE
  never parks on the ScalarE exp cadence.
- The last attention row's outputs are transposed on the PE (via identity
  matmul) instead of the DRAM round-trip, removing the final transpose DMA
  latency from the critical path.
- A dummy warmup matmul chain absorbs the PE p-state ramp while the first
  input DMA pieces land; the first projection chunk runs et-outer so the PE
  starts consuming pieces as they arrive.
- Output is written fp16 and summed on host in fp32.
"""

import sys
import time

sys.path.insert(0, "/opt/trn_rl_repo")

from collections import deque

import numpy as np

import concourse.bass as bass
import concourse.mybir as mybir
import concourse.tile as tile
from concourse import bacc
from concourse.bass_utils import run_bass_kernel_spmd

B, N, E, H = 2, 2048, 2048, 16
D = E // H            # 128
HPC = 4               # heads per core
DC = HPC * D          # 512 head dims per core
NCORES = 8
P = 128
NCH = N // 512        # 4 n-chunks of 512
ET = E // P           # 16 e-tiles of 128

F32 = mybir.dt.float32
FP16 = mybir.dt.float16


def build_nc():
    nc = bacc.Bacc("TRN2", target_bir_lowering=False, debug=False,
                   num_devices=NCORES)

    xT = nc.dram_tensor("xT", [E, N], FP16, kind="ExternalInput")
    wqT = nc.dram_tensor("wqT", [E, DC], FP16, kind="ExternalInput")
    wkT = nc.dram_tensor("wkT", [E, DC], FP16, kind="ExternalInput")
    wvT = nc.dram_tensor("wvT", [E, DC], FP16, kind="ExternalInput")
    woT = nc.dram_tensor("woT", [DC, E], FP16, kind="ExternalInput")
    maskin = nc.dram_tensor("maskin", [P, 2, P], FP16, kind="ExternalInput")
    out = nc.dram_tensor("out", [N, E], FP16, kind="ExternalOutput")

    xT_r = xT.ap().rearrange("(eo p) n -> p eo n", p=P)      # [128,16,2048]
    wqT_r = wqT.ap().rearrange("(eo p) d -> p eo d", p=P)    # [128,16,512]
    wkT_r = wkT.ap().rearrange("(eo p) d -> p eo d", p=P)
    wvT_r = wvT.ap().rearrange("(eo p) d -> p eo d", p=P)
    woT_r = woT.ap().rearrange("(t p) e -> p t e", p=P)      # [128,4,2048]

    EXPF = mybir.ActivationFunctionType.Exp

    with tile.TileContext(nc) as tc:
        # ---------------- constants + spill tensors ----------------
        consts = tc.alloc_tile_pool(name="consts", bufs=1)
        _longlived = [consts]
        mask_sb = consts.tile([P, 2, P], FP16)   # [tri(c>=p) | identity]
        warm_sb = consts.tile([P, 512], FP16)
        # prefire the Exp table load so it overlaps the input DMA head
        dummy = consts.tile([1, 8], F32)
        nc.gpsimd.memset(warm_sb, 0.0)
        nc.vector.memset(dummy, 0.0)
        nc.scalar.activation(out=dummy, in_=dummy, func=EXPF)

        dram = tc.alloc_tile_pool(name="dram", bufs=1, space="DRAM")
        _longlived.append(dram)
        attd = dram.tile([HPC, N, D], FP16)          # normalized attn out

        # per-core activations, SBUF-resident across the whole kernel
        big = tc.alloc_tile_pool(name="big", bufs=1)
        _longlived.append(big)
        qs = big.tile([P, HPC, N], FP16)                  # q^T, heads stacked
        ks = big.tile([P, HPC, N], FP16)                  # k^T
        v_all = big.tile([P, N // P, HPC, D + 4], FP16)   # [V | 1] per block
        nc.vector.memset(v_all[:, :, :, D:D + 1], 1.0)

        outT_pool = tc.alloc_tile_pool(name="outT", bufs=1)
        _longlived.append(outT_pool)
        outTs = [outT_pool.tile([P, N], FP16, name=f"outT{t}")
                 for t in range(HPC)]
        wo_pool = tc.alloc_tile_pool(name="wo_pool", bufs=1)
        _longlived.append(wo_pool)
        wo_sb = wo_pool.tile([P, HPC, E], FP16)

        pt_pool = tc.alloc_tile_pool(name="pt_pool", bufs=2)
        att_pool = tc.alloc_tile_pool(name="att_pool", bufs=3)
        rs_pool = tc.alloc_tile_pool(name="rs_pool", bufs=8)

        # ---------------- phase 1a: nch-0 projections (et-outer) --------
        wpool = tc.alloc_tile_pool(name="wpool", bufs=1)
        xpool = tc.alloc_tile_pool(name="xpool", bufs=2)
        wq_sb = wpool.tile([P, ET, DC], FP16)
        wk_sb = wpool.tile([P, ET, DC], FP16)
        wv_sb = wpool.tile([P, ET, DC], FP16)
        x_tiles = [None] * NCH

        def load_x(nch):
            t = xpool.tile([P, ET, 512], FP16, tag="xchunk",
                           name=f"x_sb{nch}")
            nc.sync.dma_start(
                out=t, in_=xT_r[:, :, nch * 512:(nch + 1) * 512])
            x_tiles[nch] = t

        warmp = tc.alloc_tile_pool(name="warmp", bufs=1, space="PSUM")
        pj8 = tc.alloc_tile_pool(name="pj8", bufs=4, space="PSUM")
        qkw = tc.alloc_tile_pool(name="qkw", bufs=1, space="PSUM")
        avw = tc.alloc_tile_pool(name="avw", bufs=1, space="PSUM")
        if True:
            # PE warmup: absorb the p-state ramp on dummy matmuls while the
            # first input pieces stream in
            warm_ps = warmp.tile([P, 512], F32, tag="warm")
            for w in range(7):
                nc.tensor.matmul(warm_ps, lhsT=warm_sb[:, 0:P],
                                 rhs=warm_sb, start=(w == 0), stop=(w == 6))

            # small pieces throughout: PE consumption (~0.85us/et) only just
            # trails DMA supply (~0.72us/et), so a late big piece stalls PE
            x0 = xpool.tile([P, ET, 512], FP16, tag="xchunk", name="x_sb0")
            x_tiles[0] = x0
            for a, b in ((0, 1), (1, 2), (2, 4), (4, 6), (6, 8), (8, 10),
                         (10, 12), (12, 14), (14, 16)):
                gs = slice(a, b)
                nc.sync.dma_start(out=wq_sb[:, gs, :], in_=wqT_r[:, gs, :])
                nc.sync.dma_start(out=x0[:, gs, :], in_=xT_r[:, gs, 0:512])
            for g in range(4):
                gs = slice(g * 4, (g + 1) * 4)
                nc.sync.dma_start(out=wk_sb[:, gs, :], in_=wkT_r[:, gs, :])
            load_x(1)
            nc.sync.dma_start(out=mask_sb, in_=maskin.ap())
            HF = ET // 2
            nc.sync.dma_start(out=wv_sb[:, :HF, :], in_=wvT_r[:, :HF, :])
            nc.sync.dma_start(out=wv_sb[:, HF:, :], in_=wvT_r[:, HF:, :])

            for w_sb, dst in ((wq_sb, qs), (wk_sb, ks)):
                pss = [pj8.tile([P, 512], F32, tag="pjps", name=f"pjt{t}")
                       for t in range(HPC)]
                for et in range(ET):
                    for t in range(HPC):
                        nc.tensor.matmul(
                            pss[t],
                            lhsT=w_sb[:, et, t * P:(t + 1) * P],
                            rhs=x0[:, et, :],
                            start=(et == 0), stop=(et == ET - 1),
                        )
                for t in range(HPC):
                    nc.vector.tensor_copy(out=dst[:, t, 0:512], in_=pss[t])
            for nb in range(4):
                ps = pj8.tile([P, 512], F32, tag="pjps")
                for et in range(ET):
                    nc.tensor.matmul(
                        ps,
                        lhsT=x0[:, et, nb * P:(nb + 1) * P],
                        rhs=wv_sb[:, et, :],
                        start=(et == 0), stop=(et == ET - 1),
                    )
                nc.vector.tensor_copy(
                    out=v_all[:, nb, :, :D],
                    in_=ps.rearrange("p (h d) -> p h d", h=HPC))
                if nb == 2:
                    # first nch-1 q chain here: its PE work hides the last
                    # v eviction that gates the next pool's coarse sems
                    ps_q1 = pj8.tile([P, 512], F32, tag="pjps",
                                     name="psq1")
                    for et in range(ET):
                        nc.tensor.matmul(
                            ps_q1,
                            lhsT=wq_sb[:, et, 0:P],
                            rhs=x_tiles[1][:, et, :],
                            start=(et == 0), stop=(et == ET - 1),
                        )
                    nc.vector.tensor_copy(out=qs[:, 0, 512:1024],
                                          in_=ps_q1)
            load_x(2)
            # wo loads here (DMA slack mid-proj) so the attention-phase DMA
            # queue is free for the attention spills/transposes
            for t in range(HPC):
                nc.sync.dma_start(out=wo_sb[:, t, :], in_=woT_r[:, t, :])

        # ------- phase 1b + 2: proj nch 1-3 (with ci=0 attn tiles woven
        # in), then attention rows 1-3 merged with the out-projection -----
        if True:
            psum_pools = {}
            tri = mask_sb[:, 0, :]
            ident = mask_sb[:, 1, :]

            # ---- attention tile helpers (used for ci=0 during proj and
            # for rows 1..3 in the attention phase) ----
            def emit_qk_pair(ci, h, pt, pi):
                npairs = 2 * ci + 2
                bj0, bj1 = 2 * pi, 2 * pi + 1
                ps = psum_pools["qk"].tile([P, 2, 512], F32, tag="qkps")
                for u, bj in ((0, bj0), (1, bj1)):
                    rr = bj - 4 * ci
                    if rr <= 0:
                        nc.tensor.matmul(
                            ps[:, u, :],
                            lhsT=ks[:, h, bj * P:(bj + 1) * P],
                            rhs=qs[:, h, ci * 512:(ci + 1) * 512],
                            start=True, stop=True,
                        )
                    else:
                        nc.tensor.matmul(
                            ps[:, u, rr * P:],
                            lhsT=ks[:, h, bj * P:(bj + 1) * P],
                            rhs=qs[:, h, ci * 512 + rr * P:(ci + 1) * 512],
                            start=True, stop=True,
                        )
                if pi == npairs - 1:
                    # diagonal pair: only causally-valid columns
                    nc.scalar.activation(
                        out=pt[:, bj0, 256:], in_=ps[:, 0, 256:], func=EXPF)
                    nc.scalar.activation(
                        out=pt[:, bj1, 384:], in_=ps[:, 1, 384:], func=EXPF)
                else:
                    nc.scalar.activation(
                        out=pt[:, bj0:bj0 + 2, :], in_=ps, func=EXPF)
                # triangular mask on strictly-diagonal 128x128 squares
                for u, bj in ((0, bj0), (1, bj1)):
                    rr = bj - 4 * ci
                    if rr >= 0:
                        sq = slice(rr * P, (rr + 1) * P)
                        nc.vector.tensor_mul(
                            out=pt[:, bj, sq], in0=pt[:, bj, sq], in1=tri)

            def av_item(ci, h, ib, pt, att_h):
                gi = 4 * ci + ib
                avp = psum_pools["av"].tile([P, D + 4], F32, tag="avps")
                isl = slice(ib * P, (ib + 1) * P)
                for bj in range(gi + 1):
                    nc.tensor.matmul(
                        avp[:, :D + 1],
                        lhsT=pt[:, bj, isl],
                        rhs=v_all[:, bj, h, :D + 1],
                        start=(bj == 0), stop=(bj == gi),
                    )
                rs = rs_pool.tile([P, 1], F32, tag="rs")
                nc.vector.reciprocal_approx_fast(out=rs, in_=avp[:, D:D + 1])
                nc.vector.tensor_scalar_mul(
                    out=att_h[:, ib, :], in0=avp[:, :D], scalar1=rs)
                if h == HPC - 1 and ci > 0:
                    # the last head's transpose gates the whole out-proj
                    # group: do it on the PE (identity matmul) so the group
                    # unlocks right after the eviction
                    tp = psum_pools["av"].tile([P, D], FP16, tag="avps",
                                               name="tps")
                    nc.tensor.transpose(tp, att_h[:, ib, :], ident)
                    nc.vector.tensor_copy(
                        out=outTs[h][:, ci * 512 + ib * P:
                                     ci * 512 + (ib + 1) * P],
                        in_=tp)
                elif ib == 3:
                    # non-gating heads: DRAM round-trip transpose (zero PE
                    # cost); lands several tiles before the group unlocks
                    nsl = slice(ci * 512, (ci + 1) * 512)
                    nc.sync.dma_start(
                        out=attd[h, nsl, :].rearrange(
                            "(io p) d -> p io d", p=P),
                        in_=att_h)
                    nc.sync.dma_start_transpose(
                        out=outTs[h][:, nsl], in_=attd[h, nsl, :])

            # ---- proj nch 1..3 with ci=0 tiles woven between chains ----
            if True:
                psum_pools["qk"] = qkw
                psum_pools["av"] = avw
                pts0 = [pt_pool.tile([P, ET, 512], FP16, tag="pt",
                                     name=f"pt0{h}") for h in range(HPC)]
                atts0 = [att_pool.tile([P, NCH, D], FP16, tag="atth",
                                       name=f"att0{h}") for h in range(HPC)]
                # (kind, args): proj chain steps interleaved with ci=0 work
                weave = deque()
                for h in range(HPC):
                    weave.append(("qk", h, 0))
                    weave.append(("qk", h, 1))
                    for ib in range(4):
                        weave.append(("av", h, ib))

                def weave_step(budget):
                    # pop ci=0 attn pieces; each is tiny vs a proj chain
                    n = 0
                    while weave and n < budget:
                        kind, h, idx = weave[0]
                        if kind == "qk":
                            emit_qk_pair(0, h, pts0[h], idx)
                        else:
                            av_item(0, h, idx, pts0[h], atts0[h])
                        weave.popleft()
                        n += 1

                for nch in range(1, NCH):
                    x_sb = x_tiles[nch]
                    nsl = slice(nch * 512, (nch + 1) * 512)

                    for w_sb, dst in ((wq_sb, qs), (wk_sb, ks)):
                        for t in range(HPC):
                            if nch == 1 and w_sb is wq_sb and t == 0:
                                continue
                            ps = pj8.tile([P, 512], F32, tag="pjps")
                            for et in range(ET):
                                nc.tensor.matmul(
                                    ps,
                                    lhsT=w_sb[:, et, t * P:(t + 1) * P],
                                    rhs=x_sb[:, et, :],
                                    start=(et == 0), stop=(et == ET - 1),
                                )
                            nc.vector.tensor_copy(
                                out=dst[:, t, nsl], in_=ps)
                            weave_step(1)

                    for nb in range(4):
                        ps = pj8.tile([P, 512], F32, tag="pjps")
                        for et in range(ET):
                            nc.tensor.matmul(
                                ps,
                                lhsT=x_sb[:, et, nb * P:(nb + 1) * P],
                                rhs=wv_sb[:, et, :],
                                start=(et == 0), stop=(et == ET - 1),
                            )
                        nc.vector.tensor_copy(
                            out=v_all[:, nch * 4 + nb, :, :D],
                            in_=ps.rearrange("p (h d) -> p h d", h=HPC))
                        weave_step(1)

                    if nch + 2 < NCH:
                        load_x(nch + 2)
                weave_step(99)
            avw.release()
            qkw.release()
            pj8.release()
            warmp.release()
            xpool.release()
            wpool.release()

            # ---- attention rows 1..3 + out-projection fill ----
            with (
                tc.tile_pool(name="qk_ps", bufs=2, space="PSUM") as qk_ps,
                tc.tile_pool(name="av_ps", bufs=2, space="PSUM") as av_ps,
                tc.tile_pool(name="op_ps", bufs=2, space="PSUM") as op_ps,
                tc.tile_pool(name="op_ev", bufs=3) as op_ev,
            ):
                psum_pools["qk"] = qk_ps
                psum_pools["av"] = av_ps
                favq = deque()     # (cost_ns, emit_fn) A@V of the prev tile
                fopq = deque()     # (cost_ns, emit_fn) out-proj items
                transposed = [HPC, 0, 0, 0]
                tdone_step = [-99, None, None, None]
                op_queued = [False] * NCH
                op_state = {}      # nb -> ostage tile
                in_drain = [False]
                dcnt = [0]
                evict_flip = [0]
                step = [0]

                def make_op_item(nb, ec):
                    def emit():
                        if ec == 0:
                            op_state[nb] = op_ev.tile(
                                [P, NCH, 512], FP16, tag="opev",
                                name=f"ost{nb}")
                        ostage = op_state[nb]
                        if in_drain[0]:
                            dcnt[0] += 1
                            if dcnt[0] % 2 == 0:
                                ps = psum_pools["av"].tile(
                                    [P, 512], F32, tag="avps", name="opalt")
                            else:
                                ps = op_ps.tile([P, 512], F32, tag="opps")
                        else:
                            ps = op_ps.tile([P, 512], F32, tag="opps")
                        for t in range(HPC):
                            nc.tensor.matmul(
                                ps,
                                lhsT=outTs[t][:, nb * P:(nb + 1) * P],
                                rhs=wo_sb[:, t, ec * 512:(ec + 1) * 512],
                                start=(t == 0), stop=(t == HPC - 1),
                            )
                        nc.vector.tensor_copy(out=ostage[:, ec, :],
                                              in_=ps)
                        if nb == 4 * NCH - 1:
                            # very last row-block: per-ec DMAs so the final
                            # transfer trailing the last matmul is small
                            nc.sync.dma_start(
                                out=out.ap()[nb * P:(nb + 1) * P,
                                             ec * 512:(ec + 1) * 512],
                                in_=ostage[:, ec, :])
                        elif nb >= 4 * (NCH - 1):
                            if ec == 1:
                                nc.sync.dma_start(
                                    out=out.ap()[nb * P:(nb + 1) * P,
                                                 0:1024],
                                    in_=ostage[:, 0:2, :])
                            elif ec == 3:
                                nc.sync.dma_start(
                                    out=out.ap()[nb * P:(nb + 1) * P,
                                                 1024:2048],
                                    in_=ostage[:, 2:4, :])
                        elif ec == NCH - 1:
                            nc.sync.dma_start(
                                out=out.ap()[nb * P:(nb + 1) * P, :],
                                in_=ostage)
                    return emit

                def queue_ready_op():
                    for cig in range(NCH):
                        if op_queued[cig] or transposed[cig] < HPC:
                            continue
                        if cig > 0 and step[0] < tdone_step[cig] + 1:
                            continue
                        op_queued[cig] = True
                        for nb in range(cig * 4, cig * 4 + 4):
                            for ec in range(NCH):
                                fopq.append((4 * 213, make_op_item(nb, ec)))

                def emit_fill(target_ns, prefer_op):
                    acc = 0
                    while acc < target_ns:
                        if prefer_op and fopq:
                            q = fopq
                        elif favq:
                            q = favq
                        elif fopq:
                            q = fopq
                        else:
                            return
                        cost, fn = q.popleft()
                        fn()
                        acc += cost
                        prefer_op = False

                def mark_transposed(ci):
                    transposed[ci] += 1
                    if transposed[ci] == HPC:
                        tdone_step[ci] = step[0]

                prev = None  # (ci, h, pt, att_h)
                for ci in range(1, NCH):
                    for h in range(HPC):
                        queue_ready_op()
                        if prev is not None:
                            pci, ph, ppt, patt = prev
                            for ib in range(4):
                                def mk(pci=pci, ph=ph, ib=ib, ppt=ppt,
                                       patt=patt):
                                    def em():
                                        av_item(pci, ph, ib, ppt, patt)
                                        if ib == 3:
                                            mark_transposed(pci)
                                    return em
                                favq.append(
                                    ((4 * pci + ib + 1) * 54 + 150, mk()))
                        pt = pt_pool.tile([P, ET, 512], FP16, tag="pt")
                        att_h = att_pool.tile([P, NCH, D], FP16, tag="atth")
                        npairs = 2 * ci + 2
                        for pi in range(npairs):
                            # fill BEFORE the pair: the pair's PSUM bank is
                            # gated by an earlier pair's exp, and the PE is
                            # in-order — fill emitted after a stalled matmul
                            # would be stuck behind it
                            emit_fill(
                                400 if pi == npairs - 1 else 800,
                                prefer_op=(pi == 0))
                            emit_qk_pair(ci, h, pt, pi)
                        prev = (ci, h, pt, att_h)
                        step[0] += 1

                # drain: last tile's A@V with PE-side transposes, then the
                # remaining out-proj chunks
                pci, ph, ppt, patt = prev
                while favq:
                    favq.popleft()[1]()
                for ib in range(4):
                    av_item(pci, ph, ib, ppt, patt)
                transposed[pci] = HPC
                tdone_step[pci] = step[0] - 2
                in_drain[0] = True
                queue_ready_op()
                while fopq:
                    fopq.popleft()[1]()
                step[0] += 4
                queue_ready_op()
                while fopq:
                    fopq.popleft()[1]()

        rs_pool.release()
        att_pool.release()
        pt_pool.release()
        for _pl in reversed(_longlived):
            _pl.release()

    nc.compile()
    return nc


def make_in_maps(x, Wq, Wkv, Wout):
    x = np.asarray(x, dtype=np.float32)
    Wq = np.asarray(Wq, dtype=np.float32)
    Wkv = np.asarray(Wkv, dtype=np.float32)
    Wout = np.asarray(Wout, dtype=np.float32)
    scale = np.float32(D ** -0.5)

    # [strictly-diagonal causal mask (col >= row) | identity]
    jj = np.arange(P)[:, None]
    ii = np.arange(P)[None, :]
    mask = np.zeros((P, 2, P), dtype=np.float16)
    mask[:, 0, :] = (ii >= jj).astype(np.float16)
    mask[:, 1, :] = (ii == jj).astype(np.float16)

    xT = [np.ascontiguousarray(x[b].T).astype(np.float16) for b in range(B)]
    in_maps = []
    for c in range(NCORES):
        b, hg = divmod(c, 4)
        sl = slice(hg * DC, (hg + 1) * DC)
        in_maps.append({
            "xT": xT[b],
            "wqT": (np.ascontiguousarray(Wq[sl, :].T) * scale).astype(np.float16),
            "wkT": np.ascontiguousarray(Wkv[sl, :].T).astype(np.float16),
            "wvT": np.ascontiguousarray(Wkv[E + sl.start:E + sl.stop, :].T).astype(np.float16),
            "woT": np.ascontiguousarray(Wout[:, sl].T).astype(np.float16),
            "maskin": mask,
        })
    return in_maps


_NC_CACHE = []


def _get_nc():
    if not _NC_CACHE:
        _NC_CACHE.append(build_nc())
    return _NC_CACHE[0]


def _run(in_maps):
    nc = _get_nc()
    return run_bass_kernel_spmd(nc, in_maps, core_ids=list(range(NCORES)))


def kernel(x, Wq, Wkv, Wout):
    in_maps = make_in_maps(x, Wq, Wkv, Wout)
    res = _run(in_maps)
    out = np.zeros((B, N, E), dtype=np.float32)
    for c in range(NCORES):
        out[c // 4] += res.results[c]["out"].astype(np.float32)
    return out


if __name__ == "__main__":
    t0 = time.time()
    _get_nc()
    print(f"build+compile: {time.time() - t0:.1f}s")


# revision 43
# speedup vs baseline: 1.1000x; 1.0017x over previous
"""Trainium2 Bass kernel for nn_BaseAttention (B=2, N=2048, E=2048, H=16, D=128).

Sharding: 8 cores; core c handles batch b=c//4, head-group hg=c%4 (4 heads).
Each core computes q/k/v projections for its heads, causal flash-style
attention, and a partial out-projection (contraction over its 512 head dims).
Host sums the 4 partial outputs per batch (tensor-parallel unshard).

Schedule (v3):
- QK^T and A@V are causally exact at 128-block granularity; only the
  strictly-diagonal 128x128 squares get a triangular mask multiply.
- The four ci=0 attention tiles (which need only chunk 0 of q/k/v) are woven
  into the projection phase, so their exp/spill/transpose chain finishes long
  before the attention phase starts.
- In the attention phase, out-projection matmuls and the previous tile's A@V
  are emitted as PE fill work BEFORE each QK PSUM pair, so the in-order PE
  never parks on the ScalarE exp cadence.
- The last attention row's outputs are transposed on the PE (via identity
  matmul) instead of the DRAM round-trip, removing the final transpose DMA
  latency from the critical path.
- A dummy warmup matmul chain absorbs the PE p-state ramp while the first
  input DMA pieces land; the first projection chunk runs et-outer so the PE
  starts consuming pieces as they arrive.
- Output is written fp16 and summed on host in fp32.
"""

import sys
import time

sys.path.insert(0, "/opt/trn_rl_repo")

from collections import deque

import numpy as np

import concourse.bass as bass
import concourse.mybir as mybir
import concourse.tile as tile
from concourse import bacc
from concourse.bass_utils import run_bass_kernel_spmd

B, N, E, H = 2, 2048, 2048, 16
D = E // H            # 128
HPC = 4               # heads per core
DC = HPC * D          # 512 head dims per core
NCORES = 8
P = 128
NCH = N // 512        # 4 n-chunks of 512
ET = E // P           # 16 e-tiles of 128

F32 = mybir.dt.float32
FP16 = mybir.dt.float16


def build_nc():
    nc = bacc.Bacc("TRN2", target_bir_lowering=False, debug=False,
                   num_devices=NCORES)

    xT = nc.dram_tensor("xT", [E, N], FP16, kind="ExternalInput")
    wqT = nc.dram_tensor("wqT", [E, DC], FP16, kind="ExternalInput")
    wkT = nc.dram_tensor("wkT", [E, DC], FP16, kind="ExternalInput")
    wvT = nc.dram_tensor("wvT", [E, DC], FP16, kind="ExternalInput")
    woT = nc.dram_tensor("woT", [DC, E], FP16, kind="ExternalInput")
    maskin = nc.dram_tensor("maskin", [P, 2, P], FP16, kind="ExternalInput")
    out = nc.dram_tensor("out", [N, E], FP16, kind="ExternalOutput")

    xT_r = xT.ap().rearrange("(eo p) n -> p eo n", p=P)      # [128,16,2048]
    wqT_r = wqT.ap().rearrange("(eo p) d -> p eo d", p=P)    # [128,16,512]
    wkT_r = wkT.ap().rearrange("(eo p) d -> p eo d", p=P)
    wvT_r = wvT.ap().rearrange("(eo p) d -> p eo d", p=P)
    woT_r = woT.ap().rearrange("(t p) e -> p t e", p=P)      # [128,4,2048]

    EXPF = mybir.ActivationFunctionType.Exp

    with tile.TileContext(nc) as tc:
        # ---------------- constants + spill tensors ----------------
        consts = tc.alloc_tile_pool(name="consts", bufs=1)
        _longlived = [consts]
        mask_sb = consts.tile([P, 2, P], FP16)   # [tri(c>=p) | identity]
        warm_sb = consts.tile([P, 512], FP16)
        # prefire the Exp table load so it overlaps the input DMA head
        dummy = consts.tile([1, 8], F32)
        nc.gpsimd.memset(warm_sb, 0.0)
        nc.vector.memset(dummy, 0.0)
        nc.scalar.activation(out=dummy, in_=dummy, func=EXPF)

        dram = tc.alloc_tile_pool(name="dram", bufs=1, space="DRAM")
        _longlived.append(dram)
        attd = dram.tile([HPC, N, D], FP16)          # normalized attn out

        # per-core activations, SBUF-resident across the whole kernel
        big = tc.alloc_tile_pool(name="big", bufs=1)
        _longlived.append(big)
        qs = big.tile([P, HPC, N], FP16)                  # q^T, heads stacked
        ks = big.tile([P, HPC, N], FP16)                  # k^T
        v_all = big.tile([P, N // P, HPC, D + 4], FP16)   # [V | 1] per block
        nc.vector.memset(v_all[:, :, :, D:D + 1], 1.0)

        outT_pool = tc.alloc_tile_pool(name="outT", bufs=1)
        _longlived.append(outT_pool)
        outTs = [outT_pool.tile([P, N], FP16, name=f"outT{t}")
                 for t in range(HPC)]
        wo_pool = tc.alloc_tile_pool(name="wo_pool", bufs=1)
        _longlived.append(wo_pool)
        wo_sb = wo_pool.tile([P, HPC, E], FP16)

        pt_pool = tc.alloc_tile_pool(name="pt_pool", bufs=2)
        att_pool = tc.alloc_tile_pool(name="att_pool", bufs=3)
        rs_pool = tc.alloc_tile_pool(name="rs_pool", bufs=8)

        # ---------------- phase 1a: nch-0 projections (et-outer) --------
        wpool = tc.alloc_tile_pool(name="wpool", bufs=1)
        xpool = tc.alloc_tile_pool(name="xpool", bufs=2)
        wq_sb = wpool.tile([P, ET, DC], FP16)
        wk_sb = wpool.tile([P, ET, DC], FP16)
        wv_sb = wpool.tile([P, ET, DC], FP16)
        x_tiles = [None] * NCH

        def load_x(nch):
            t = xpool.tile([P, ET, 512], FP16, tag="xchunk",
                           name=f"x_sb{nch}")
            nc.sync.dma_start(
                out=t, in_=xT_r[:, :, nch * 512:(nch + 1) * 512])
            x_tiles[nch] = t

        warmp = tc.alloc_tile_pool(name="warmp", bufs=1, space="PSUM")
        pj8 = tc.alloc_tile_pool(name="pj8", bufs=4, space="PSUM")
        qkw = tc.alloc_tile_pool(name="qkw", bufs=1, space="PSUM")
        avw = tc.alloc_tile_pool(name="avw", bufs=1, space="PSUM")
        if True:
            # PE warmup: absorb the p-state ramp on dummy matmuls while the
            # first input pieces stream in
            warm_ps = warmp.tile([P, 512], F32, tag="warm")
            for w in range(7):
                nc.tensor.matmul(warm_ps, lhsT=warm_sb[:, 0:P],
                                 rhs=warm_sb, start=(w == 0), stop=(w == 6))

            # small pieces throughout: PE consumption (~0.85us/et) only just
            # trails DMA supply (~0.72us/et), so a late big piece stalls PE
            x0 = xpool.tile([P, ET, 512], FP16, tag="xchunk", name="x_sb0")
            x_tiles[0] = x0
            for a, b in ((0, 1), (1, 2), (2, 4), (4, 6), (6, 8), (8, 10),
                         (10, 12), (12, 14), (14, 16)):
                gs = slice(a, b)
                nc.sync.dma_start(out=wq_sb[:, gs, :], in_=wqT_r[:, gs, :])
                nc.sync.dma_start(out=x0[:, gs, :], in_=xT_r[:, gs, 0:512])
            for g in range(4):
                gs = slice(g * 4, (g + 1) * 4)
                nc.sync.dma_start(out=wk_sb[:, gs, :], in_=wkT_r[:, gs, :])
            load_x(1)
            nc.sync.dma_start(out=mask_sb, in_=maskin.ap())
            HF = ET // 2
            nc.sync.dma_start(out=wv_sb[:, :HF, :], in_=wvT_r[:, :HF, :])
            nc.sync.dma_start(out=wv_sb[:, HF:, :], in_=wvT_r[:, HF:, :])

            for w_sb, dst in ((wq_sb, qs), (wk_sb, ks)):
                if w_sb is wk_sb:
                    # k chain 0 borrows the (idle until nch-1) weave QK pool
                    # so it doesn't wait on q chain 0's eviction
                    p0 = qkw.tile([P, 512], F32, tag="qkps", name="kb0")
                    pss = [p0] + [
                        pj8.tile([P, 512], F32, tag="pjps", name=f"pjt{t}")
                        for t in range(1, HPC)]
                else:
                    pss = [pj8.tile([P, 512], F32, tag="pjps",
                                    name=f"pjt{t}") for t in range(HPC)]
                for et in range(ET):
                    for t in range(HPC):
                        nc.tensor.matmul(
                            pss[t],
                            lhsT=w_sb[:, et, t * P:(t + 1) * P],
                            rhs=x0[:, et, :],
                            start=(et == 0), stop=(et == ET - 1),
                        )
                for t in range(HPC):
                    nc.vector.tensor_copy(out=dst[:, t, 0:512], in_=pss[t])
            for nb in range(4):
                if nb == 0:
                    # v chain 0 borrows the idle weave AV pool
                    ps = avw.tile([P, 512], F32, tag="avps", name="vb0")
                else:
                    ps = pj8.tile([P, 512], F32, tag="pjps")
                for et in range(ET):
                    nc.tensor.matmul(
                        ps,
                        lhsT=x0[:, et, nb * P:(nb + 1) * P],
                        rhs=wv_sb[:, et, :],
                        start=(et == 0), stop=(et == ET - 1),
                    )
                nc.vector.tensor_copy(
                    out=v_all[:, nb, :, :D],
                    in_=ps.rearrange("p (h d) -> p h d", h=HPC))
                if nb == 2:
                    # first nch-1 q chain here: its PE work hides the last
                    # v eviction that gates the next pool's coarse sems
                    ps_q1 = pj8.tile([P, 512], F32, tag="pjps",
                                     name="psq1")
                    for et in range(ET):
                        nc.tensor.matmul(
                            ps_q1,
                            lhsT=wq_sb[:, et, 0:P],
                            rhs=x_tiles[1][:, et, :],
                            start=(et == 0), stop=(et == ET - 1),
                        )
                    nc.vector.tensor_copy(out=qs[:, 0, 512:1024],
                                          in_=ps_q1)
            load_x(2)
            # wo loads here (DMA slack mid-proj) so the attention-phase DMA
            # queue is free for the attention spills/transposes
            for t in range(HPC):
                nc.sync.dma_start(out=wo_sb[:, t, :], in_=woT_r[:, t, :])

        # ------- phase 1b + 2: proj nch 1-3 (with ci=0 attn tiles woven
        # in), then attention rows 1-3 merged with the out-projection -----
        if True:
            psum_pools = {}
            tri = mask_sb[:, 0, :]
            ident = mask_sb[:, 1, :]

            # ---- attention tile helpers (used for ci=0 during proj and
            # for rows 1..3 in the attention phase) ----
            def emit_qk_pair(ci, h, pt, pi):
                npairs = 2 * ci + 2
                bj0, bj1 = 2 * pi, 2 * pi + 1
                ps = psum_pools["qk"].tile([P, 2, 512], F32, tag="qkps")
                for u, bj in ((0, bj0), (1, bj1)):
                    rr = bj - 4 * ci
                    if rr <= 0:
                        nc.tensor.matmul(
                            ps[:, u, :],
                            lhsT=ks[:, h, bj * P:(bj + 1) * P],
                            rhs=qs[:, h, ci * 512:(ci + 1) * 512],
                            start=True, stop=True,
                        )
                    else:
                        nc.tensor.matmul(
                            ps[:, u, rr * P:],
                            lhsT=ks[:, h, bj * P:(bj + 1) * P],
                            rhs=qs[:, h, ci * 512 + rr * P:(ci + 1) * 512],
                            start=True, stop=True,
                        )
                if pi == npairs - 1:
                    # diagonal pair: only causally-valid columns
                    nc.scalar.activation(
                        out=pt[:, bj0, 256:], in_=ps[:, 0, 256:], func=EXPF)
                    nc.scalar.activation(
                        out=pt[:, bj1, 384:], in_=ps[:, 1, 384:], func=EXPF)
                else:
                    nc.scalar.activation(
                        out=pt[:, bj0:bj0 + 2, :], in_=ps, func=EXPF)
                # triangular mask on strictly-diagonal 128x128 squares
                for u, bj in ((0, bj0), (1, bj1)):
                    rr = bj - 4 * ci
                    if rr >= 0:
                        sq = slice(rr * P, (rr + 1) * P)
                        nc.vector.tensor_mul(
                            out=pt[:, bj, sq], in0=pt[:, bj, sq], in1=tri)

            def av_item(ci, h, ib, pt, att_h):
                gi = 4 * ci + ib
                avp = psum_pools["av"].tile([P, D + 4], F32, tag="avps")
                isl = slice(ib * P, (ib + 1) * P)
                for bj in range(gi + 1):
                    nc.tensor.matmul(
                        avp[:, :D + 1],
                        lhsT=pt[:, bj, isl],
                        rhs=v_all[:, bj, h, :D + 1],
                        start=(bj == 0), stop=(bj == gi),
                    )
                rs = rs_pool.tile([P, 1], F32, tag="rs")
                nc.vector.reciprocal_approx_fast(out=rs, in_=avp[:, D:D + 1])
                nc.vector.tensor_scalar_mul(
                    out=att_h[:, ib, :], in0=avp[:, :D], scalar1=rs)
                if h == HPC - 1 and ci > 0:
                    # the last head's transpose gates the whole out-proj
                    # group: do it on the PE (identity matmul) so the group
                    # unlocks right after the eviction
                    tp = psum_pools["av"].tile([P, D], FP16, tag="avps",
                                               name="tps")
                    nc.tensor.transpose(tp, att_h[:, ib, :], ident)
                    nc.vector.tensor_copy(
                        out=outTs[h][:, ci * 512 + ib * P:
                                     ci * 512 + (ib + 1) * P],
                        in_=tp)
                elif ib == 3:
                    # non-gating heads: DRAM round-trip transpose (zero PE
                    # cost); lands several tiles before the group unlocks
                    nsl = slice(ci * 512, (ci + 1) * 512)
                    nc.sync.dma_start(
                        out=attd[h, nsl, :].rearrange(
                            "(io p) d -> p io d", p=P),
                        in_=att_h)
                    nc.sync.dma_start_transpose(
                        out=outTs[h][:, nsl], in_=attd[h, nsl, :])

            # ---- proj nch 1..3 with ci=0 tiles woven between chains ----
            if True:
                psum_pools["qk"] = qkw
                psum_pools["av"] = avw
                pts0 = [pt_pool.tile([P, ET, 512], FP16, tag="pt",
                                     name=f"pt0{h}") for h in range(HPC)]
                atts0 = [att_pool.tile([P, NCH, D], FP16, tag="atth",
                                       name=f"att0{h}") for h in range(HPC)]
                # (kind, args): proj chain steps interleaved with ci=0 work
                weave = deque()
                for h in range(HPC):
                    weave.append(("qk", h, 0))
                    weave.append(("qk", h, 1))
                    for ib in range(4):
                        weave.append(("av", h, ib))

                def weave_step(budget):
                    # pop ci=0 attn pieces; each is tiny vs a proj chain
                    n = 0
                    while weave and n < budget:
                        kind, h, idx = weave[0]
                        if kind == "qk":
                            emit_qk_pair(0, h, pts0[h], idx)
                        else:
                            av_item(0, h, idx, pts0[h], atts0[h])
                        weave.popleft()
                        n += 1

                for nch in range(1, NCH):
                    x_sb = x_tiles[nch]
                    nsl = slice(nch * 512, (nch + 1) * 512)

                    for w_sb, dst in ((wq_sb, qs), (wk_sb, ks)):
                        for t in range(HPC):
                            if nch == 1 and w_sb is wq_sb and t == 0:
                                continue
                            ps = pj8.tile([P, 512], F32, tag="pjps")
                            for et in range(ET):
                                nc.tensor.matmul(
                                    ps,
                                    lhsT=w_sb[:, et, t * P:(t + 1) * P],
                                    rhs=x_sb[:, et, :],
                                    start=(et == 0), stop=(et == ET - 1),
                                )
                            nc.vector.tensor_copy(
                                out=dst[:, t, nsl], in_=ps)
                            weave_step(1)

                    for nb in range(4):
                        ps = pj8.tile([P, 512], F32, tag="pjps")
                        for et in range(ET):
                            nc.tensor.matmul(
                                ps,
                                lhsT=x_sb[:, et, nb * P:(nb + 1) * P],
                                rhs=wv_sb[:, et, :],
                                start=(et == 0), stop=(et == ET - 1),
                            )
                        nc.vector.tensor_copy(
                            out=v_all[:, nch * 4 + nb, :, :D],
                            in_=ps.rearrange("p (h d) -> p h d", h=HPC))
                        weave_step(1)

                    if nch + 2 < NCH:
                        load_x(nch + 2)
                weave_step(99)
            avw.release()
            qkw.release()
            pj8.release()
            warmp.release()
            xpool.release()
            wpool.release()

            # ---- attention rows 1..3 + out-projection fill ----
            with (
                tc.tile_pool(name="qk_ps", bufs=2, space="PSUM") as qk_ps,
                tc.tile_pool(name="av_ps", bufs=2, space="PSUM") as av_ps,
                tc.tile_pool(name="op_ps", bufs=2, space="PSUM") as op_ps,
                tc.tile_pool(name="op_ev", bufs=3) as op_ev,
            ):
                psum_pools["qk"] = qk_ps
                psum_pools["av"] = av_ps
                favq = deque()     # (cost_ns, emit_fn) A@V of the prev tile
                fopq = deque()     # (cost_ns, emit_fn) out-proj items
                transposed = [HPC, 0, 0, 0]
                tdone_step = [-99, None, None, None]
                op_queued = [False] * NCH
                op_state = {}      # nb -> ostage tile
                in_drain = [False]
                dcnt = [0]
                evict_flip = [0]
                step = [0]

                def make_op_item(nb, ec):
                    def emit():
                        if ec == 0:
                            op_state[nb] = op_ev.tile(
                                [P, NCH, 512], FP16, tag="opev",
                                name=f"ost{nb}")
                        ostage = op_state[nb]
                        if in_drain[0]:
                            dcnt[0] += 1
                            if dcnt[0] % 2 == 0:
                                ps = psum_pools["av"].tile(
                                    [P, 512], F32, tag="avps", name="opalt")
                            else:
                                ps = op_ps.tile([P, 512], F32, tag="opps")
                        else:
                            ps = op_ps.tile([P, 512], F32, tag="opps")
                        for t in range(HPC):
                            nc.tensor.matmul(
                                ps,
                                lhsT=outTs[t][:, nb * P:(nb + 1) * P],
                                rhs=wo_sb[:, t, ec * 512:(ec + 1) * 512],
                                start=(t == 0), stop=(t == HPC - 1),
                            )
                        nc.vector.tensor_copy(out=ostage[:, ec, :],
                                              in_=ps)
                        if nb == 4 * NCH - 1:
                            # very last row-block: per-ec DMAs so the final
                            # transfer trailing the last matmul is small
                            nc.sync.dma_start(
                                out=out.ap()[nb * P:(nb + 1) * P,
                                             ec * 512:(ec + 1) * 512],
                                in_=ostage[:, ec, :])
                        elif nb >= 4 * (NCH - 1):
                            if ec == 1:
                                nc.sync.dma_start(
                                    out=out.ap()[nb * P:(nb + 1) * P,
                                                 0:1024],
                                    in_=ostage[:, 0:2, :])
                            elif ec == 3:
                                nc.sync.dma_start(
                                    out=out.ap()[nb * P:(nb + 1) * P,
                                                 1024:2048],
                                    in_=ostage[:, 2:4, :])
                        elif ec == NCH - 1:
                            nc.sync.dma_start(
                                out=out.ap()[nb * P:(nb + 1) * P, :],
                                in_=ostage)
                    return emit

                def queue_ready_op():
                    for cig in range(NCH):
                        if op_queued[cig] or transposed[cig] < HPC:
                            continue
                        if cig > 0 and step[0] < tdone_step[cig] + 1:
                            continue
                        op_queued[cig] = True
                        for nb in range(cig * 4, cig * 4 + 4):
                            for ec in range(NCH):
                                fopq.append((4 * 213, make_op_item(nb, ec)))

                def emit_fill(target_ns, prefer_op):
                    acc = 0
                    while acc < target_ns:
                        if prefer_op and fopq:
                            q = fopq
                        elif favq:
                            q = favq
                        elif fopq:
                            q = fopq
                        else:
                            return
                        cost, fn = q.popleft()
                        fn()
                        acc += cost
                        prefer_op = False

                def mark_transposed(ci):
                    transposed[ci] += 1
                    if transposed[ci] == HPC:
                        tdone_step[ci] = step[0]

                prev = None  # (ci, h, pt, att_h)
                for ci in range(1, NCH):
                    for h in range(HPC):
                        queue_ready_op()
                        if prev is not None:
                            pci, ph, ppt, patt = prev
                            for ib in range(4):
                                def mk(pci=pci, ph=ph, ib=ib, ppt=ppt,
                                       patt=patt):
                                    def em():
                                        av_item(pci, ph, ib, ppt, patt)
                                        if ib == 3:
                                            mark_transposed(pci)
                                    return em
                                favq.append(
                                    ((4 * pci + ib + 1) * 54 + 150, mk()))
                        pt = pt_pool.tile([P, ET, 512], FP16, tag="pt")
                        att_h = att_pool.tile([P, NCH, D], FP16, tag="atth")
                        npairs = 2 * ci + 2
                        for pi in range(npairs):
                            # fill BEFORE the pair: the pair's PSUM bank is
                            # gated by an earlier pair's exp, and the PE is
                            # in-order — fill emitted after a stalled matmul
                            # would be stuck behind it
                            emit_fill(
                                400 if pi == npairs - 1 else 800,
                                prefer_op=(pi == 0))
                            emit_qk_pair(ci, h, pt, pi)
                        prev = (ci, h, pt, att_h)
                        step[0] += 1

                # drain: last tile's A@V with PE-side transposes, then the
                # remaining out-proj chunks
                pci, ph, ppt, patt = prev
                while favq:
                    favq.popleft()[1]()
                for ib in range(4):
                    av_item(pci, ph, ib, ppt, patt)
                transposed[pci] = HPC
                tdone_step[pci] = step[0] - 2
                in_drain[0] = True
                queue_ready_op()
                while fopq:
                    fopq.popleft()[1]()
                step[0] += 4
                queue_ready_op()
                while fopq:
                    fopq.popleft()[1]()

        rs_pool.release()
        att_pool.release()
        pt_pool.release()
        for _pl in reversed(_longlived):
            _pl.release()

    nc.compile()
    return nc


def make_in_maps(x, Wq, Wkv, Wout):
    x = np.asarray(x, dtype=np.float32)
    Wq = np.asarray(Wq, dtype=np.float32)
    Wkv = np.asarray(Wkv, dtype=np.float32)
    Wout = np.asarray(Wout, dtype=np.float32)
    scale = np.float32(D ** -0.5)

    # [strictly-diagonal causal mask (col >= row) | identity]
    jj = np.arange(P)[:, None]
    ii = np.arange(P)[None, :]
    mask = np.zeros((P, 2, P), dtype=np.float16)
    mask[:, 0, :] = (ii >= jj).astype(np.float16)
    mask[:, 1, :] = (ii == jj).astype(np.float16)

    xT = [np.ascontiguousarray(x[b].T).astype(np.float16) for b in range(B)]
    in_maps = []
    for c in range(NCORES):
        b, hg = divmod(c, 4)
        sl = slice(hg * DC, (hg + 1) * DC)
        in_maps.append({
            "xT": xT[b],
            "wqT": (np.ascontiguousarray(Wq[sl, :].T) * scale).astype(np.float16),
            "wkT": np.ascontiguousarray(Wkv[sl, :].T).astype(np.float16),
            "wvT": np.ascontiguousarray(Wkv[E + sl.start:E + sl.stop, :].T).astype(np.float16),
            "woT": np.ascontiguousarray(Wout[:, sl].T).astype(np.float16),
            "maskin": mask,
        })
    return in_maps


_NC_CACHE = []


def _get_nc():
    if not _NC_CACHE:
        _NC_CACHE.append(build_nc())
    return _NC_CACHE[0]


def _run(in_maps):
    nc = _get_nc()
    return run_bass_kernel_spmd(nc, in_maps, core_ids=list(range(NCORES)))


def kernel(x, Wq, Wkv, Wout):
    in_maps = make_in_maps(x, Wq, Wkv, Wout)
    res = _run(in_maps)
    out = np.zeros((B, N, E), dtype=np.float32)
    for c in range(NCORES):
        out[c // 4] += res.results[c]["out"].astype(np.float32)
    return out


if __name__ == "__main__":
    t0 = time.time()
    _get_nc()
    print(f"build+compile: {time.time() - t0:.1f}s")
